# revision 27
# baseline (speedup 1.0000x reference)
"""Trainium2 Bass kernel for nn_FCN8sAtOnceMultiGnn2 (gnn_message_passing).

Strategy (8 NeuronCores; sample s = core//2, node-half = core%2):
  The GNN messages only feed a per-(sample,channel) SE gate: m_r/m_i are
  consumed by a full mean over nodes, so per iteration we only need
    S[c] = sum_edges lrelu(P[r_e,c] - Q[q_e,c] + b_c)
  where P/Q are per-sample tables h @ W (h = gate-scaled pooled features).
  The final output is relu(g1*prod(gate)*rgb_pooled + g2*prod(1-gate)*ir_pooled).

  Per core: maxpool -> bf16 Gram -> top-16 via DVE max8/max_index/match_replace
  -> edge lists -> per iteration: scale weights by accumulated gate products,
  compute combined tables T_r=[Wr1+Wr2 | Wi2] (rgb nodes), T_q=[Wr2 | Wi1+Wi2]
  (ir nodes) on the PE (+bias), cast fp8, write each to its own DRAM tensor
  (so a-side gathers only depend on T_r and can start while T_q is still
  emitting), dma_gather rows at the edge indices, d = sub (DVE/GPSIMD per a
  static schedule), |d| = Abs (ACT/DVE), reduce per channel with fp8
  DoubleRow ones-matmuls on PE accumulating in PSUM
  (lrelu sum = .505*sum(d)+.495*sum|d|), pairwise AllReduce the [2,512]
  partial sums, SE MLP -> gate. Host reassembles halves.
"""
import sys

sys.path.insert(0, "/opt/trn_rl_repo")

import numpy as np

_CACHE = {}

P = 128
C = 512          # channels
NT = 1024        # nodes per sample (32*32 after pool)
HN = 512         # nodes per core (half sample)
KNN = 16
E = HN * KNN     # 8192 edges per core per direction
NCH = 8          # gather chunks per iteration (per side)
ECH = E // NCH   # 1024 edge indices per gather
N_CORES = 8
LOOKAHEAD = 2    # chunks of gather issued ahead of elementwise work

# elementwise engine schedule: 16 units per iteration = (chunk, dirn)
# unit id u = ch*2 + dirn
POOL_SUB_UNITS = frozenset({5, 9, 13})          # subs on GPSIMD (rest DVE)
POOL_ABS_UNITS = frozenset()                    # (tensor_scalar invalid on Pool)
DVE_ABS_UNITS = frozenset()                     # (abs_max tensor_scalar is
                                                #  not a valid real-DVE op)


def _build(iterations: int, timing: bool = False):
    from contextlib import ExitStack

    import concourse.bacc as bacc
    import concourse.bass as bass
    import concourse.mybir as mybir
    import concourse.tile as tile

    dt = mybir.dt
    f32, bf16, i16, u16, f8 = (dt.float32, dt.bfloat16, dt.int16, dt.uint16,
                               dt.float8e4)
    AF = mybir.ActivationFunctionType
    OP = mybir.AluOpType
    SC_LIN = 0.505 / float(NT * KNN)
    SC_ABS = 0.495 / float(NT * KNN)

    nc = bacc.Bacc("TRN2", target_bir_lowering=False, debug=False,
                   num_devices=1 if timing else N_CORES)

    rgb_in = nc.dram_tensor("rgb", [C, 64, 64], f32, kind="ExternalInput")
    ir_in = nc.dram_tensor("ir", [C, 64, 64], f32, kind="ExternalInput")
    wrgb_in = nc.dram_tensor("wrgb", [2 * C, C], f32, kind="ExternalInput")
    wir_in = nc.dram_tensor("wir", [2 * C, C], f32, kind="ExternalInput")
    brgb_in = nc.dram_tensor("brgb", [1, C], f32, kind="ExternalInput")
    bir_in = nc.dram_tensor("bir", [1, C], f32, kind="ExternalInput")
    wse1_in = nc.dram_tensor("wse1", [2 * C, 32], f32, kind="ExternalInput")
    bse1_in = nc.dram_tensor("bse1", [1, 32], f32, kind="ExternalInput")
    wse2_in = nc.dram_tensor("wse2", [32, C], f32, kind="ExternalInput")
    bse2_in = nc.dram_tensor("bse2", [1, C], f32, kind="ExternalInput")
    g1_in = nc.dram_tensor("g1", [1, 1], f32, kind="ExternalInput")
    g2_in = nc.dram_tensor("g2", [1, 1], f32, kind="ExternalInput")
    out_t = nc.dram_tensor("out", [C, HN], f32, kind="ExternalOutput")

    MODS = ("r", "i")
    mod_in = {"r": rgb_in, "i": ir_in}

    with tile.TileContext(nc) as tc:
        with (
            tc.tile_pool(name="persist", bufs=1) as pp,
            tc.tile_pool(name="big", bufs=2) as bigp,
            tc.tile_pool(name="dram", bufs=1, space="DRAM") as dram,
        ):
            # ---------------- constants / persistent tiles ----------------
            ones_bf = pp.tile([P, 1], bf16, tag="ones_bf")
            nc.vector.memset(ones_bf[:], 1.0)
            ones64 = pp.tile([P, 2, 64], f8, tag="ones64")
            nc.vector.memset(ones64[:], 1.0)
            ones_row = pp.tile([1, P], f32, tag="ones_row")
            nc.vector.memset(ones_row[:], 1.0)
            ones_row_bf = pp.tile([1, P], bf16, tag="ones_row_bf")
            nc.vector.memset(ones_row_bf[:], 1.0)

            xb = {m: [pp.tile([P, NT], bf16, tag=f"xb_{m}{cc}", name=f"xb_{m}{cc}")
                      for cc in range(4)] for m in MODS}
            phalf = {m: [pp.tile([P, HN], bf16, tag=f"ph_{m}{cc}", name=f"ph_{m}{cc}")
                         for cc in range(4)] for m in MODS}
            rn = {m: pp.tile([1, NT], f32, tag=f"rn_{m}", name=f"rn_{m}")
                  for m in MODS}
            Bn = {m: pp.tile([P, NT], bf16, tag=f"Bn_{m}", name=f"Bn_{m}")
                  for m in MODS}
            idx_mt = {m: [pp.tile([P, KNN], u16, tag=f"ix_{m}{t}", name=f"ix_{m}{t}")
                          for t in range(4)] for m in MODS}
            eidx3 = pp.tile([P, NCH, 128], i16, tag="eix", name="eix")
            Wc = {"r": pp.tile([P, 4, 2 * C], bf16, tag="Wc_r", name="Wc_r"),
                  "q": pp.tile([P, 4, 2 * C], bf16, tag="Wc_q", name="Wc_q")}
            # bias rows for the PE bias-matmul: r-table biases cols 0:C,
            # q-table biases cols C:2C; the other half has zero bias
            brow = {"r": pp.tile([1, C], bf16, tag="brow_r", name="brow_r"),
                    "q": pp.tile([1, C], bf16, tag="brow_q", name="brow_q")}
            wse1_sb = pp.tile([P, 8, 32], f32, tag="wse1", name="wse1")
            bse1_sb = pp.tile([32, 1], f32, tag="bse1", name="bse1")
            wse2_sb = pp.tile([32, C], f32, tag="wse2", name="wse2")
            bse2_sb = pp.tile([P, 4], f32, tag="bse2", name="bse2")
            gb = {1: pp.tile([P, 1], f32, tag="gb1", name="gb1"),
                  2: pp.tile([P, 1], f32, tag="gb2", name="gb2")}
            a_r = pp.tile([P, 4], f32, tag="a_r", name="a_r")
            a_i = pp.tile([P, 4], f32, tag="a_i", name="a_i")
            nc.vector.memset(a_r[:], 1.0)
            nc.vector.memset(a_i[:], 1.0)

            # ---------------- weights / SE / bias prep ----------------
            def emit_weights_prep():
                with (
                    tc.tile_pool(name="s4", bufs=1) as s4,
                    tc.tile_pool(name="ps_c", bufs=2, space="PSUM") as ps_c_p,
                ):
                    def wload(dst, src_t, lohi):
                        nc.sync.dma_start(
                            dst[:],
                            src_t[lohi * C:(lohi + 1) * C, :].rearrange(
                                "(k p) c -> p k c", p=P))
                    wa = s4.tile([P, 4, C], f32, tag="wa", name="wa")
                    wb = s4.tile([P, 4, C], f32, tag="wb", name="wb")
                    wload(wa, wrgb_in, 1)
                    wload(wb, wrgb_in, 0)
                    for k in range(4):
                        nc.vector.tensor_tensor(out=Wc["r"][:, k, 0:C],
                                                in0=wb[:, k, :],
                                                in1=wa[:, k, :], op=OP.add)
                        nc.vector.tensor_copy(Wc["q"][:, k, 0:C], wa[:, k, :])
                    wb2 = s4.tile([P, 4, C], f32, tag="wb", name="wb2")
                    wload(wb2, wir_in, 1)
                    for k in range(4):
                        nc.vector.tensor_copy(Wc["r"][:, k, C:2 * C],
                                              wb2[:, k, :])
                    wa2 = s4.tile([P, 4, C], f32, tag="wa", name="wa2")
                    wload(wa2, wir_in, 0)
                    for k in range(4):
                        nc.vector.tensor_tensor(out=Wc["q"][:, k, C:2 * C],
                                                in0=wa2[:, k, :],
                                                in1=wb2[:, k, :], op=OP.add)
                    brow32 = s4.tile([1, C], f32, tag="brow32", name="brow32")
                    for nm, src_b in (("r", brgb_in), ("q", bir_in)):
                        nc.sync.dma_start(brow32[:], src_b[:])
                        nc.vector.tensor_copy(brow[nm][:], brow32[:])
                    nc.sync.dma_start(
                        wse1_sb[:],
                        wse1_in[:].rearrange("(k p) n -> p k n", p=P))

                    nc.sync.dma_start(bse1_sb[:], bse1_in[:].rearrange("a b -> b a"))
                    nc.sync.dma_start(wse2_sb[:], wse2_in[:])
                    nc.sync.dma_start(
                        bse2_sb[:],
                        bse2_in[:].rearrange("one (c p) -> (one p) c", p=P))
                    for gi, gsrc in ((1, g1_in), (2, g2_in)):
                        grow = s4.tile([1, 1], f32, tag="grow", name="grow")
                        nc.sync.dma_start(grow[:], gsrc[:])
                        psg2 = ps_c_p.tile([P, 1], f32, space="PSUM", tag="psg2",
                                           name="psg2")
                        nc.tensor.matmul(psg2[:], ones_row[:], grow[:],
                                         start=True, stop=True)
                        nc.vector.tensor_copy(gb[gi][:], psg2[:])

            # ---------------- stage 1 (per modality) ----------------
            drow_d_map = {}
            it0_ctx = ExitStack()
            ps_it0 = it0_ctx.enter_context(
                tc.tile_pool(name="psit0", bufs=2, space="PSUM"))
            s1_ctx = ExitStack()
            s1 = s1_ctx.enter_context(tc.tile_pool(name="s1", bufs=1))
            ps_ss_p = s1_ctx.enter_context(
                tc.tile_pool(name="ps_ss", bufs=1, space="PSUM"))

            ps_ss_map = {}

            def stage1_mod(m, ccs=(0, 1, 2, 3), tail=True):
                if m not in ps_ss_map:
                    ps_ss_map[m] = [ps_ss_p.tile([1, C], f32, space="PSUM",
                                                 tag=f"ss{h}",
                                                 name=f"ss{m}{h}")
                                    for h in range(2)]
                ps_ss = ps_ss_map[m]
                for cc in ccs:
                    raw = s1.tile([P, 64, 64], f32, tag="raw", name="raw",
                                  bufs=2)
                    nc.sync.dma_start(raw[:], mod_in[m][cc * P:(cc + 1) * P])
                    h1 = s1.tile([P, 32, 64], f32, tag="h1", name="h1",
                                 bufs=2)
                    nc.vector.tensor_tensor(out=h1[:], in0=raw[:, 0::2, :],
                                            in1=raw[:, 1::2, :], op=OP.max)
                    pf = s1.tile([P, 32, 32], f32, tag="pf", name="pf",
                                 bufs=2)
                    nc.vector.tensor_tensor(out=pf[:], in0=h1[:, :, 0::2],
                                            in1=h1[:, :, 1::2], op=OP.max)
                    pff = pf.rearrange("p a b -> p (a b)")
                    nc.scalar.activation(xb[m][cc][:], pff, AF.Copy)
                    nc.scalar.activation(phalf[m][cc][:], pff[:, 0:HN], AF.Copy)
                    sq = s1.tile([P, NT], bf16, tag="sq", name="sq", bufs=2)
                    nc.scalar.activation(sq[:], pff, AF.Square)
                    for h in range(2):
                        nc.tensor.matmul(ps_ss[h][:], ones_bf[:],
                                         sq[:, h * C:(h + 1) * C],
                                         start=(cc == 0), stop=(cc == 3))
                if not tail:
                    return
                srow = s1.tile([1, NT], f32, tag="srow", name="srow")
                for h in range(2):
                    nc.scalar.activation(srow[:, h * C:(h + 1) * C],
                                         ps_ss[h][:], AF.Sqrt)
                nc.vector.tensor_scalar_max(srow[:], srow[:], 1e-12)
                nc.vector.reciprocal(rn[m][:], srow[:])
                rnb = s1.tile([1, NT], bf16, tag="rnb", name="rnb")
                nc.vector.tensor_copy(rnb[:], rn[m][:])
                for h in range(2):
                    psb = ps_ss_p.tile([P, C], f32, space="PSUM", tag="psbn",
                                       name=f"psbn_{m}{h}", bufs=2)
                    nc.tensor.matmul(psb[:], ones_row_bf[:],
                                     rnb[:, h * C:(h + 1) * C],
                                     start=True, stop=True)
                    nc.scalar.activation(Bn[m][:, h * C:(h + 1) * C],
                                         psb[:], AF.Copy)

            # ---------------- per-iteration phases ----------------
            xsrc = {"r": xb["r"], "q": xb["i"]}

            def emit_table(it, tb, td, ps_it):
                bj = 0 if tb == "r" else 1  # column half that carries bias
                for i in range(8):
                    tst8 = bigp.tile([P, 2 * C], f8, tag="tst",
                                     name="tst8", bufs=4)
                    for j in range(2):
                        pst = ps_it.tile([P, C], f32, space="PSUM",
                                         tag="pst", name="pst", bufs=2)
                        for k in range(4):
                            nc.tensor.matmul(
                                pst[:],
                                xsrc[tb][k][:, i * P:(i + 1) * P],
                                Wc[tb][:, k, j * C:(j + 1) * C],
                                start=(k == 0), stop=(k == 3 and j != bj))
                        if j == bj:
                            nc.tensor.matmul(
                                pst[:], ones_row_bf[:],
                                brow[tb][:], start=False, stop=True,
                                skip_group_check=True)
                        # PSUM -> fp8 casts split across ACT and DVE
                        if j == 0:
                            nc.scalar.activation(
                                tst8[:, j * C:(j + 1) * C], pst[:], AF.Copy)
                        else:
                            nc.vector.tensor_copy(
                                tst8[:, j * C:(j + 1) * C], pst[:])
                    nc.sync.dma_start(td[i * P:(i + 1) * P, :], tst8[:])

            def gather_phase(it, ictx, ps_it, td_r, td_q):
                dap = ictx.enter_context(
                    tc.tile_pool(name=f"dabs{it}", bufs=4))
                psS_p = ictx.enter_context(
                    tc.tile_pool(name=f"psS{it}", bufs=1, space="PSUM"))
                ps_S = {q: psS_p.tile([64, C], f32, space="PSUM",
                                      tag=f"S{q}", name=f"S{q}_{it}")
                        for q in ("lin_r", "abs_r", "lin_i", "abs_i")}
                gt = {}

                def issue(ch):
                    g1t = dap.tile([P, 8, 2 * C], f8, tag="g1",
                                   name="g1t", bufs=5)
                    nc.gpsimd.dma_gather(
                        out_ap=g1t[:], in_ap=td_r[:],
                        idxs_ap=eidx3[:, ch, 0:64],
                        num_idxs=ECH, num_idxs_reg=ECH,
                        elem_size=2 * C)
                    g2t = dap.tile([P, 8, 2 * C], f8, tag="g2",
                                   name="g2t", bufs=5)
                    nc.gpsimd.dma_gather(
                        out_ap=g2t[:], in_ap=td_q[:],
                        idxs_ap=eidx3[:, ch, 64:128],
                        num_idxs=ECH, num_idxs_reg=ECH,
                        elem_size=2 * C)
                    gt[ch] = (g1t, g2t)

                # per-direction counts for PSUM accumulation start/stop flags
                n_emitted = {"lin_r": 0, "lin_i": 0, "abs_r": 0, "abs_i": 0}
                deferred = []

                def mm_reduce(kind, dirn, src):
                    k = n_emitted[f"{kind}_{dirn}"]
                    n_emitted[f"{kind}_{dirn}"] += 1
                    first = k == 0
                    last = k == NCH - 1
                    for s in range(4):
                        nc.tensor.matmul(
                            ps_S[f"{kind}_{dirn}"][:], ones64[:],
                            src[:, 2 * s:2 * s + 2, :],
                            start=(first and s == 0),
                            stop=(last and s == 3),
                            perf_mode=mybir.MatmulPerfMode.DoubleRow,
                            skip_group_check=True)

                def sub_part(dirn, ga, gbuf, lo, pool):
                    tg = "p" if pool else ""
                    dd = dap.tile([P, 8, C], f8, tag=f"{tg}dd",
                                  name="dd", bufs=4 if pool else 3)
                    seng = nc.gpsimd if pool else nc.vector
                    seng.tensor_tensor(
                        out=dd[:], in0=ga[:, :, lo:lo + C],
                        in1=gbuf[:, :, lo:lo + C], op=OP.subtract)
                    # fp8 DoubleRow: two edge rows per matmul; rows of
                    # the 64-row psum hold identical sums (read row 0)
                    mm_reduce("lin", dirn, dd)
                    return dd

                def abs_part(dirn, dd, pool):
                    tg = "p" if pool else ""
                    ad = dap.tile([P, 8, C], f8, tag=f"{tg}ad",
                                  name="ad", bufs=2)
                    if pool:
                        nc.gpsimd.tensor_scalar(
                            ad[:], dd[:], 0.0, None, op0=OP.abs_max)
                    else:
                        nc.scalar.activation(ad[:], dd[:], AF.Abs)
                    mm_reduce("abs", dirn, ad)

                for ch in range(min(LOOKAHEAD + 1, NCH)):
                    issue(ch)
                for ch in range(NCH):
                    g1t, g2t = gt[ch]
                    for di, (dirn, ga, gbuf, lo) in enumerate(
                            (("r", g1t, g2t, 0), ("i", g2t, g1t, C))):
                        u = ch * 2 + di
                        # GPSIMD work is deferred to the program tail so it
                        # never sits ahead of later desc-gen in Pool's queue
                        if u in POOL_SUB_UNITS:
                            deferred.append(("subabs", dirn, ga, gbuf, lo))
                        else:
                            dd = sub_part(dirn, ga, gbuf, lo, False)
                            if u in POOL_ABS_UNITS:
                                deferred.append(("abs", dirn, dd, None, None))
                            else:
                                abs_part(dirn, dd, False)
                    nxt = ch + LOOKAHEAD + 1
                    if nxt < NCH:
                        issue(nxt)
                for kind, dirn, a, b, lo in deferred:
                    if kind == "subabs":
                        dd = sub_part(dirn, a, b, lo, True)
                        abs_part(dirn, dd, False)
                    else:
                        abs_part(dirn, a, True)
                # S rows, AllReduce, chunked readback
                arin = dram.tile([2, C], f32, tag=f"arin{it}", name=f"arin{it}")
                arout = dram.tile([2, C], f32, tag=f"arout{it}",
                                  name=f"arout{it}")
                for row, dirn in ((0, "r"), (1, "i")):
                    t1r = dap.tile([1, C], f32, tag="t1r", name="t1r")
                    t2r = dap.tile([1, C], f32, tag="t2r", name="t2r")
                    nc.vector.tensor_scalar(t1r[:],
                                            ps_S[f"lin_{dirn}"][0:1, :],
                                            SC_LIN, None, op0=OP.mult)
                    nc.vector.tensor_scalar(t2r[:],
                                            ps_S[f"abs_{dirn}"][0:1, :],
                                            SC_ABS, None, op0=OP.mult)
                    nc.vector.tensor_tensor(out=t1r[:], in0=t1r[:], in1=t2r[:],
                                            op=OP.add)
                    nc.sync.dma_start(arin[row:row + 1, :], t1r[:])
                if timing:
                    nc.gpsimd.dma_start(arout[:], arin[:])
                else:
                    nc.gpsimd.collective_compute(
                        "AllReduce", OP.add,
                        replica_groups=[[0, 1], [2, 3], [4, 5], [6, 7]],
                        ins=[arin.opt()], outs=[arout.opt()])
                cS = dap.tile([P, 8], f32, tag="cS", name="cS")
                for row in range(2):
                    nc.sync.dma_start(
                        cS[:, row * 4:(row + 1) * 4],
                        arout[row:row + 1, :].rearrange(
                            "one (c p) -> (one p) c", p=P))
                # SE MLP
                with tc.tile_pool(name=f"ps_se{it}", bufs=1,
                                  space="PSUM") as ps_se_p:
                    ps_h1 = ps_se_p.tile([32, 1], f32, space="PSUM",
                                         tag="ps_h1", name="ps_h1")
                    for j in range(8):
                        nc.tensor.matmul(ps_h1[:], wse1_sb[:, j, :],
                                         cS[:, j:j + 1],
                                         start=(j == 0), stop=(j == 7))
                    h1r = dap.tile([32, 1], f32, tag="h1r", name="h1r")
                    nc.vector.tensor_tensor(out=h1r[:], in0=ps_h1[:],
                                            in1=bse1_sb[:], op=OP.add)
                    h1b = dap.tile([32, 1], f32, tag="h1b", name="h1b")
                    nc.vector.tensor_scalar_mul(h1b[:], h1r[:], 0.01)
                    nc.vector.tensor_tensor(out=h1r[:], in0=h1r[:], in1=h1b[:],
                                            op=OP.max)
                    ps_gate = ps_se_p.tile([P, 4], f32, space="PSUM",
                                           tag="ps_gate", name="ps_gate")
                    for j in range(4):
                        nc.tensor.matmul(ps_gate[:, j:j + 1],
                                         wse2_sb[:, j * P:(j + 1) * P],
                                         h1r[:], start=True, stop=True,
                                         skip_group_check=True)
                    gpre = dap.tile([P, 4], f32, tag="gpre", name="gpre")
                    nc.vector.tensor_tensor(out=gpre[:], in0=ps_gate[:],
                                            in1=bse2_sb[:], op=OP.add)
                    gate = dap.tile([P, 4], f32, tag="gate", name="gate")
                    nc.scalar.activation(gate[:], gpre[:], AF.Sigmoid)
                    nc.vector.tensor_tensor(out=a_r[:], in0=a_r[:], in1=gate[:],
                                            op=OP.mult)
                    omg = dap.tile([P, 4], f32, tag="omg", name="omg")
                    nc.vector.tensor_scalar(omg[:], gate[:], -1.0, 1.0,
                                            op0=OP.mult, op1=OP.add)
                    nc.vector.tensor_tensor(out=a_i[:], in0=a_i[:], in1=omg[:],
                                            op=OP.mult)
                    # fold the new gate into the combined weights in place:
                    # W_t+1 = gate_t (x) W_t along the contraction channels
                    for tb, gv in (("r", gate), ("q", omg)):
                        for k in range(4):
                            nc.vector.tensor_scalar(
                                Wc[tb][:, k, :], Wc[tb][:, k, :],
                                gv[:, k:k + 1], None, op0=OP.mult)

            # main flow: per-modality pipeline; iteration-0 tables are
            # emitted right after the modality they depend on loads, so
            # PE/DMA table work overlaps the Gram/top-k phase.
            td_r0 = dram.tile([NT, 2 * C], f8, tag="Tr0", name="Tr0")
            td_q0 = dram.tile([NT, 2 * C], f8, tag="Tq0", name="Tq0")
            exd_comb = dram.tile([1, 2 * E], u16, tag="exd", name="exd_comb")
            stage1_mod("r", ccs=(0,), tail=False)
            emit_weights_prep()
            stage1_mod("r", ccs=(1, 2, 3))
            stage1_mod("i", ccs=(0, 1), tail=False)
            emit_table(0, "r", td_r0, ps_it0)
            stage1_mod("i", ccs=(2, 3))
            emit_table(0, "q", td_q0, ps_it0)
            s1_ctx.close()
            with (
                tc.tile_pool(name="s2", bufs=2) as s2,
                tc.tile_pool(name="s2b", bufs=1) as s2b,
                tc.tile_pool(name="ps_g", bufs=2, space="PSUM") as ps_g_p,
            ):
                # normalized features: Gram of xbn is the cosine similarity,
                # so ranking by it directly equals ranking by -distance
                xbn = {m: [s2b.tile([P, NT], bf16, tag=f"xbn_{m}{k}",
                                    name=f"xbn_{m}{k}") for k in range(4)]
                       for m in MODS}
                for m in MODS:
                    for k in range(4):
                        nc.vector.tensor_tensor(out=xbn[m][k][:],
                                                in0=xb[m][k][:],
                                                in1=Bn[m][:], op=OP.mult)
                for t in range(4):
                    for m in MODS:
                        ab = 0 if m == "r" else 1
                        nd = s2.tile([P, NT], f32, tag="nd", name="nd")
                        for h in range(2):
                            psg = ps_g_p.tile([P, C], f32, space="PSUM",
                                              tag="psg", name="psg")
                            for k in range(4):
                                nc.tensor.matmul(
                                    psg[:],
                                    xbn[m][k][:, t * P:(t + 1) * P],
                                    xbn[m][k][:, h * C:(h + 1) * C],
                                    start=(k == 0), stop=(k == 3))
                            nc.scalar.activation(nd[:, h * C:(h + 1) * C],
                                                 psg[:], AF.Copy)
                        mx = s2.tile([P, 16], f32, tag="mx", name="mx")
                        nc.vector.max(out=mx[:, 0:8], in_=nd[:])
                        nc.vector.max_index(out=idx_mt[m][t][:, 0:8],
                                            in_max=mx[:, 0:8], in_values=nd[:])
                        nc.vector.match_replace(out=nd[:],
                                                in_to_replace=mx[:, 0:8],
                                                in_values=nd[:],
                                                imm_value=-1e30)
                        nc.vector.max(out=mx[:, 8:16], in_=nd[:])
                        nc.vector.max_index(out=idx_mt[m][t][:, 8:16],
                                            in_max=mx[:, 8:16], in_values=nd[:])
                        # stage the tile's edge list: chunks 2t/2t+1, side ab
                        moff = ab * 1024
                        for hf in range(2):
                            chn = 2 * t + hf
                            base = chn * 2048 + moff
                            dst = exd_comb[0:1, base:base + 1024].rearrange(
                                "one (p k) -> (one p) k", p=64)
                            nc.sync.dma_start(
                                dst, idx_mt[m][t][hf * 64:(hf + 1) * 64, :])
                    # replicated idx stripes for chunks 2t, 2t+1
                    srcidx = exd_comb[0:1, t * 4096:(t + 1) * 4096].bitcast(
                        i16).rearrange("one (c q) -> (one q) c", q=16)
                    for s8 in range(8):
                        nc.sync.dma_start(
                            eidx3[s8 * 16:(s8 + 1) * 16, 2 * t:2 * t + 2, :],
                            srcidx)

            gather_phase(0, it0_ctx, ps_it0, td_r0, td_q0)
            it0_ctx.close()
            for it in range(1, iterations):
                ictx = ExitStack()
                ps_it = ictx.enter_context(
                    tc.tile_pool(name=f"psit{it}", bufs=2, space="PSUM"))
                td_r = dram.tile([NT, 2 * C], f8, tag=f"Tr{it}",
                                 name=f"Tr{it}")
                td_q = dram.tile([NT, 2 * C], f8, tag=f"Tq{it}",
                                 name=f"Tq{it}")
                emit_table(it, "r", td_r, ps_it)
                emit_table(it, "q", td_q, ps_it)
                gather_phase(it, ictx, ps_it, td_r, td_q)
                ictx.close()

            # ---------------- output ----------------
            with tc.tile_pool(name="s6", bufs=2) as s6:
                alpha = s6.tile([P, 4], f32, tag="alpha", name="alpha")
                beta = s6.tile([P, 4], f32, tag="beta", name="beta")
                nc.vector.tensor_scalar(alpha[:], a_r[:], gb[1][:, 0:1], None,
                                        op0=OP.mult)
                nc.vector.tensor_scalar(beta[:], a_i[:], gb[2][:, 0:1], None,
                                        op0=OP.mult)
                for cc in range(4):
                    t1 = s6.tile([P, HN], f32, tag="t1", name="t1")
                    t2 = s6.tile([P, HN], f32, tag="t2", name="t2")
                    nc.vector.tensor_scalar(t1[:], phalf["r"][cc][:],
                                            alpha[:, cc:cc + 1], None,
                                            op0=OP.mult)
                    nc.vector.scalar_tensor_tensor(
                        out=t2[:], in0=phalf["i"][cc][:],
                        scalar=beta[:, cc:cc + 1], in1=t1[:],
                        op0=OP.mult, op1=OP.add)
                    nc.vector.tensor_scalar_max(t2[:], t2[:], 0.0)
                    nc.sync.dma_start(out_t[cc * P:(cc + 1) * P, :], t2[:])

    nc.compile()
    return nc


def _prepare_in_maps(rgb, ir, W_rgb_g, b_rgb_g, W_ir_g, b_ir_g,
                     W_se1, b_se1, W_se2, b_se2, gamma1, gamma2):
    f32 = np.float32
    common = {
        "wrgb": np.ascontiguousarray(W_rgb_g, f32),
        "wir": np.ascontiguousarray(W_ir_g, f32),
        "brgb": np.ascontiguousarray(b_rgb_g, f32).reshape(1, C),
        "bir": np.ascontiguousarray(b_ir_g, f32).reshape(1, C),
        "wse1": np.ascontiguousarray(W_se1, f32),
        "bse1": np.ascontiguousarray(b_se1, f32).reshape(1, 32),
        "wse2": np.ascontiguousarray(W_se2, f32),
        "bse2": np.ascontiguousarray(b_se2, f32).reshape(1, C),
        "g1": np.asarray(gamma1, f32).reshape(1, 1),
        "g2": np.asarray(gamma2, f32).reshape(1, 1),
    }
    in_maps = []
    for core in range(N_CORES):
        s, hh = core // 2, core % 2
        r = np.asarray(rgb[s], f32)
        i = np.asarray(ir[s], f32)
        if hh:
            r = np.roll(r, -32, axis=1)
            i = np.roll(i, -32, axis=1)
        m = dict(common)
        m["rgb"] = np.ascontiguousarray(r)
        m["ir"] = np.ascontiguousarray(i)
        in_maps.append(m)
    return in_maps


def _make_runner(nc):
    """Cached replica of bass2jax.run_bass_via_pjrt's multi-core branch so
    repeated kernel() calls skip jit retracing."""
    import jax
    import concourse.mybir as mybir
    from concourse import bass2jax as b2j
    from jax.experimental.shard_map import shard_map
    from jax.sharding import Mesh, PartitionSpec

    b2j.install_neuronx_cc_hook()

    partition_name = (nc.partition_id_tensor.name
                      if nc.partition_id_tensor else None)
    in_names, out_names, out_avals, zero_outs = [], [], [], []
    for alloc in nc.m.functions[0].allocations:
        if not isinstance(alloc, mybir.MemoryLocationSet):
            continue
        name = alloc.memorylocations[0].name
        if alloc.kind == "ExternalInput":
            if name != partition_name:
                in_names.append(name)
        elif alloc.kind == "ExternalOutput":
            shape = tuple(alloc.tensor_shape)
            np_dt = mybir.dt.np(alloc.dtype)
            out_names.append(name)
            out_avals.append(jax.core.ShapedArray(shape, np_dt))
            zero_outs.append(np.zeros(shape, np_dt))

    n_params = len(in_names)
    n_outs = len(out_names)
    all_in_names = list(in_names) + list(out_names)
    if partition_name is not None:
        all_in_names.append(partition_name)
    donate = tuple(range(n_params, n_params + n_outs))

    def _body(*args):
        operands = list(args)
        if partition_name is not None:
            operands.append(b2j.partition_id_tensor())
        outs = b2j._bass_exec_p.bind(
            *operands,
            out_avals=tuple(out_avals),
            in_names=tuple(all_in_names),
            out_names=tuple(out_names),
            lowering_input_output_aliases=(),
            sim_require_finite=True,
            sim_require_nnan=True,
            nc=nc,
        )
        return tuple(outs)

    devices = jax.devices()[:N_CORES]
    mesh = Mesh(np.asarray(devices), ("core",))
    in_specs = (PartitionSpec("core"),) * (n_params + n_outs)
    out_specs = (PartitionSpec("core"),) * n_outs
    sharded = jax.jit(
        shard_map(_body, mesh=mesh, in_specs=in_specs, out_specs=out_specs,
                  check_rep=False),
        donate_argnums=donate, keep_unused=True)
    concat_zeros = [np.zeros((N_CORES * z.shape[0], *z.shape[1:]), z.dtype)
                    for z in zero_outs]

    def run(in_maps):
        concat_in = [
            np.concatenate([np.asarray(in_maps[c][nm])
                            for c in range(N_CORES)], axis=0)
            for nm in in_names
        ]
        out_arrs = sharded(*concat_in, *[z.copy() for z in concat_zeros])
        return [
            {nm: np.asarray(out_arrs[i]).reshape(
                N_CORES, *out_avals[i].shape)[c]
             for i, nm in enumerate(out_names)}
            for c in range(N_CORES)
        ]

    return run


def kernel(rgb, ir, W_rgb_g, b_rgb_g, W_ir_g, b_ir_g,
           W_se1, b_se1, W_se2, b_se2, gamma1, gamma2,
           gnn_iterations, k):
    iterations = int(gnn_iterations)
    assert int(k) == KNN, f"kernel hardcodes k=16, got {k}"
    if iterations not in _CACHE:
        nc = _build(iterations)
        _CACHE[iterations] = _make_runner(nc)
    run = _CACHE[iterations]

    in_maps = _prepare_in_maps(rgb, ir, W_rgb_g, b_rgb_g, W_ir_g, b_ir_g,
                               W_se1, b_se1, W_se2, b_se2, gamma1, gamma2)
    results = run(in_maps)

    out = np.empty((4, C, 32, 32), np.float32)
    for s in range(4):
        lo = results[2 * s]["out"].reshape(C, 16, 32)
        hi = results[2 * s + 1]["out"].reshape(C, 16, 32)
        out[s] = np.concatenate([lo, hi], axis=1)
    return out


# revision 39
# speedup vs baseline: 1.1138x; 1.1138x over previous
"""Trainium2 Bass kernel for nn_FCN8sAtOnceMultiGnn2 (gnn_message_passing).

Strategy (8 NeuronCores; sample s = core//2, node-half = core%2):
  The GNN messages only feed a per-(sample,channel) SE gate: m_r/m_i are
  consumed by a full mean over nodes, so per iteration we only need
    S[c] = sum_edges lrelu(P[r_e,c] - Q[q_e,c] + b_c)
  where P/Q are per-sample tables h @ W (h = gate-scaled pooled features).
  The final output is relu(g1*prod(gate)*rgb_pooled + g2*prod(1-gate)*ir_pooled).

  Per core: maxpool -> bf16 Gram -> top-16 via DVE max8/max_index/match_replace
  -> edge lists -> per iteration: scale weights by accumulated gate products,
  compute combined tables T_r=[Wr1+Wr2 | Wi2] (rgb nodes), T_q=[Wr2 | Wi1+Wi2]
  (ir nodes) on the PE (+bias), cast fp8, write each to its own DRAM tensor
  (so a-side gathers only depend on T_r and can start while T_q is still
  emitting), dma_gather rows at the edge indices, d = sub (DVE/GPSIMD per a
  static schedule), |d| = Abs (ACT/DVE), reduce per channel with fp8
  DoubleRow ones-matmuls on PE accumulating in PSUM
  (lrelu sum = .505*sum(d)+.495*sum|d|), pairwise AllReduce the [2,512]
  partial sums, SE MLP -> gate. Host reassembles halves.
"""
import sys

sys.path.insert(0, "/opt/trn_rl_repo")

import numpy as np

_CACHE = {}

P = 128
C = 512          # channels
NT = 1024        # nodes per sample (32*32 after pool)
HN = 512         # nodes per core (half sample)
KNN = 16
E = HN * KNN     # 8192 edges per core per direction
NCH = 8          # gather chunks per iteration (per side)
ECH = E // NCH   # 1024 edge indices per gather
N_CORES = 8
LOOKAHEAD = 2    # chunks of gather issued ahead of elementwise work

# elementwise engine schedule: 16 units per iteration = (chunk, dirn)
# unit id u = ch*2 + dirn
POOL_SUB_UNITS = frozenset({5, 9, 13})          # subs on GPSIMD (rest DVE)
POOL_ABS_UNITS = frozenset()                    # (tensor_scalar invalid on Pool)
DVE_ABS_UNITS = frozenset()                     # (abs_max tensor_scalar is
                                                #  not a valid real-DVE op)


def _build(iterations: int, timing: bool = False):
    from contextlib import ExitStack

    import concourse.bacc as bacc
    import concourse.bass as bass
    import concourse.mybir as mybir
    import concourse.tile as tile

    dt = mybir.dt
    f32, bf16, i16, u16, f8 = (dt.float32, dt.bfloat16, dt.int16, dt.uint16,
                               dt.float8e4)
    AF = mybir.ActivationFunctionType
    OP = mybir.AluOpType
    SC_LIN = 0.505 / float(NT * KNN)
    SC_ABS = 0.495 / float(NT * KNN)

    nc = bacc.Bacc("TRN2", target_bir_lowering=False, debug=False,
                   num_devices=1 if timing else N_CORES)

    rgb_in = nc.dram_tensor("rgb", [C, 64, 64], bf16, kind="ExternalInput")
    ir_in = nc.dram_tensor("ir", [C, 64, 64], bf16, kind="ExternalInput")
    wrgb_in = nc.dram_tensor("wrgb", [2 * C, C], f32, kind="ExternalInput")
    wir_in = nc.dram_tensor("wir", [2 * C, C], f32, kind="ExternalInput")
    brgb_in = nc.dram_tensor("brgb", [1, C], f32, kind="ExternalInput")
    bir_in = nc.dram_tensor("bir", [1, C], f32, kind="ExternalInput")
    wse1_in = nc.dram_tensor("wse1", [2 * C, 32], f32, kind="ExternalInput")
    bse1_in = nc.dram_tensor("bse1", [1, 32], f32, kind="ExternalInput")
    wse2_in = nc.dram_tensor("wse2", [32, C], f32, kind="ExternalInput")
    bse2_in = nc.dram_tensor("bse2", [1, C], f32, kind="ExternalInput")
    g1_in = nc.dram_tensor("g1", [1, 1], f32, kind="ExternalInput")
    g2_in = nc.dram_tensor("g2", [1, 1], f32, kind="ExternalInput")
    out_t = nc.dram_tensor("out", [C, HN], f32, kind="ExternalOutput")

    MODS = ("r", "i")
    mod_in = {"r": rgb_in, "i": ir_in}

    with tile.TileContext(nc) as tc:
        with (
            tc.tile_pool(name="persist", bufs=1) as pp,
            tc.tile_pool(name="big", bufs=2) as bigp,
            tc.tile_pool(name="dram", bufs=1, space="DRAM") as dram,
        ):
            # ---------------- constants / persistent tiles ----------------
            ones_bf = pp.tile([P, 1], bf16, tag="ones_bf")
            nc.vector.memset(ones_bf[:], 1.0)
            ones64 = pp.tile([P, 2, 64], f8, tag="ones64")
            nc.vector.memset(ones64[:], 1.0)
            ones_row = pp.tile([1, P], f32, tag="ones_row")
            nc.vector.memset(ones_row[:], 1.0)
            ones_row_bf = pp.tile([1, P], bf16, tag="ones_row_bf")
            nc.vector.memset(ones_row_bf[:], 1.0)

            xb = {m: [pp.tile([P, NT], bf16, tag=f"xb_{m}{cc}", name=f"xb_{m}{cc}")
                      for cc in range(4)] for m in MODS}
            phalf = {m: [pp.tile([P, HN], bf16, tag=f"ph_{m}{cc}", name=f"ph_{m}{cc}")
                         for cc in range(4)] for m in MODS}
            rn = {m: pp.tile([1, NT], f32, tag=f"rn_{m}", name=f"rn_{m}")
                  for m in MODS}
            Bn = {m: pp.tile([P, NT], bf16, tag=f"Bn_{m}", name=f"Bn_{m}")
                  for m in MODS}
            idx_mt = {m: [pp.tile([P, KNN], u16, tag=f"ix_{m}{t}", name=f"ix_{m}{t}")
                          for t in range(4)] for m in MODS}
            eidx_t = [pp.tile([P, 2, 128], i16, tag=f"eix{t}", name=f"eix{t}")
                      for t in range(4)]
            estg_t = [pp.tile([16, 2, 128], i16, tag=f"estg{t}",
                              name=f"estg{t}") for t in range(4)]
            Wc = {"r": pp.tile([P, 4, 2 * C], bf16, tag="Wc_r", name="Wc_r"),
                  "q": pp.tile([P, 4, 2 * C], bf16, tag="Wc_q", name="Wc_q")}
            # bias rows for the PE bias-matmul: r-table biases cols 0:C,
            # q-table biases cols C:2C; the other half has zero bias
            brow = {"r": pp.tile([1, C], bf16, tag="brow_r", name="brow_r"),
                    "q": pp.tile([1, C], bf16, tag="brow_q", name="brow_q")}
            wse1_sb = pp.tile([P, 8, 32], f32, tag="wse1", name="wse1")
            bse1_sb = pp.tile([32, 1], f32, tag="bse1", name="bse1")
            wse2_sb = pp.tile([32, C], f32, tag="wse2", name="wse2")
            bse2_sb = pp.tile([P, 4], f32, tag="bse2", name="bse2")
            gb = {1: pp.tile([P, 1], f32, tag="gb1", name="gb1"),
                  2: pp.tile([P, 1], f32, tag="gb2", name="gb2")}
            a_r = pp.tile([P, 4], f32, tag="a_r", name="a_r")
            a_i = pp.tile([P, 4], f32, tag="a_i", name="a_i")
            nc.vector.memset(a_r[:], 1.0)
            nc.vector.memset(a_i[:], 1.0)

            # ---------------- weights / SE / bias prep ----------------
            def emit_weights_prep():
                with (
                    tc.tile_pool(name="s4", bufs=1) as s4,
                    tc.tile_pool(name="ps_c", bufs=2, space="PSUM") as ps_c_p,
                ):
                    def wload(dst, src_t, lohi):
                        nc.sync.dma_start(
                            dst[:],
                            src_t[lohi * C:(lohi + 1) * C, :].rearrange(
                                "(k p) c -> p k c", p=P))
                    wa = s4.tile([P, 4, C], f32, tag="wa", name="wa")
                    wb = s4.tile([P, 4, C], f32, tag="wb", name="wb")
                    wload(wa, wrgb_in, 1)
                    wload(wb, wrgb_in, 0)
                    for k in range(4):
                        nc.gpsimd.tensor_tensor(out=Wc["r"][:, k, 0:C],
                                                in0=wb[:, k, :],
                                                in1=wa[:, k, :], op=OP.add)
                        nc.scalar.activation(Wc["q"][:, k, 0:C], wa[:, k, :],
                                             AF.Copy)
                    wb2 = s4.tile([P, 4, C], f32, tag="wb", name="wb2")
                    wload(wb2, wir_in, 1)
                    for k in range(4):
                        nc.scalar.activation(Wc["r"][:, k, C:2 * C],
                                             wb2[:, k, :], AF.Copy)
                    wa2 = s4.tile([P, 4, C], f32, tag="wa", name="wa2")
                    wload(wa2, wir_in, 0)
                    for k in range(4):
                        nc.gpsimd.tensor_tensor(out=Wc["q"][:, k, C:2 * C],
                                                in0=wa2[:, k, :],
                                                in1=wb2[:, k, :], op=OP.add)
                    brow32 = s4.tile([1, C], f32, tag="brow32", name="brow32")
                    for nm, src_b in (("r", brgb_in), ("q", bir_in)):
                        nc.sync.dma_start(brow32[:], src_b[:])
                        nc.vector.tensor_copy(brow[nm][:], brow32[:])
                    nc.sync.dma_start(
                        wse1_sb[:],
                        wse1_in[:].rearrange("(k p) n -> p k n", p=P))

                    nc.sync.dma_start(bse1_sb[:], bse1_in[:].rearrange("a b -> b a"))
                    nc.sync.dma_start(wse2_sb[:], wse2_in[:])
                    nc.sync.dma_start(
                        bse2_sb[:],
                        bse2_in[:].rearrange("one (c p) -> (one p) c", p=P))
                    for gi, gsrc in ((1, g1_in), (2, g2_in)):
                        grow = s4.tile([1, 1], f32, tag="grow", name="grow")
                        nc.sync.dma_start(grow[:], gsrc[:])
                        psg2 = ps_c_p.tile([P, 1], f32, space="PSUM", tag="psg2",
                                           name="psg2")
                        nc.tensor.matmul(psg2[:], ones_row[:], grow[:],
                                         start=True, stop=True)
                        nc.vector.tensor_copy(gb[gi][:], psg2[:])

            # ---------------- stage 1 (per modality) ----------------
            drow_d_map = {}
            it0_ctx = ExitStack()
            ps_it0 = it0_ctx.enter_context(
                tc.tile_pool(name="psit0", bufs=2, space="PSUM"))
            s1_ctx = ExitStack()
            s1 = s1_ctx.enter_context(tc.tile_pool(name="s1", bufs=1))
            ps_ss_p = s1_ctx.enter_context(
                tc.tile_pool(name="ps_ss", bufs=1, space="PSUM"))

            ps_ss_map = {}

            def stage1_mod(m, ccs=(0, 1, 2, 3), tail=True, raws=None):
                if m not in ps_ss_map:
                    ps_ss_map[m] = [ps_ss_p.tile([1, C], f32, space="PSUM",
                                                 tag=f"ss{h}",
                                                 name=f"ss{m}{h}")
                                    for h in range(2)]
                ps_ss = ps_ss_map[m]
                for cc in ccs:
                    if raws is None:
                        raw = s1.tile([P, 64, 64], bf16, tag="raw",
                                      name="raw", bufs=3)
                        nc.sync.dma_start(raw[:],
                                          mod_in[m][cc * P:(cc + 1) * P])
                    else:
                        raw = raws[cc]
                    h1 = s1.tile([P, 32, 64], bf16, tag="h1", name="h1",
                                 bufs=2)
                    nc.vector.tensor_tensor(out=h1[:], in0=raw[:, 0::2, :],
                                            in1=raw[:, 1::2, :], op=OP.max)
                    pf = s1.tile([P, 32, 32], bf16, tag="pf", name="pf",
                                 bufs=2)
                    nc.vector.tensor_tensor(out=pf[:], in0=h1[:, :, 0::2],
                                            in1=h1[:, :, 1::2], op=OP.max)
                    pff = pf.rearrange("p a b -> p (a b)")
                    nc.scalar.activation(xb[m][cc][:], pff, AF.Copy)
                    nc.scalar.activation(phalf[m][cc][:], pff[:, 0:HN], AF.Copy)
                    sq = s1.tile([P, NT], bf16, tag="sq", name="sq", bufs=2)
                    nc.scalar.activation(sq[:], pff, AF.Square)
                    for h in range(2):
                        nc.tensor.matmul(ps_ss[h][:], ones_bf[:],
                                         sq[:, h * C:(h + 1) * C],
                                         start=(cc == 0), stop=(cc == 3))
                if not tail:
                    return
                srow = s1.tile([1, NT], f32, tag="srow", name="srow")
                for h in range(2):
                    nc.scalar.activation(srow[:, h * C:(h + 1) * C],
                                         ps_ss[h][:], AF.Sqrt)
                nc.vector.tensor_scalar_max(srow[:], srow[:], 1e-12)
                nc.vector.reciprocal(rn[m][:], srow[:])
                rnb = s1.tile([1, NT], bf16, tag="rnb", name="rnb")
                nc.vector.tensor_copy(rnb[:], rn[m][:])
                for h in range(2):
                    psb = ps_ss_p.tile([P, C], f32, space="PSUM", tag="psbn",
                                       name=f"psbn_{m}{h}", bufs=2)
                    nc.tensor.matmul(psb[:], ones_row_bf[:],
                                     rnb[:, h * C:(h + 1) * C],
                                     start=True, stop=True)
                    nc.scalar.activation(Bn[m][:, h * C:(h + 1) * C],
                                         psb[:], AF.Copy)

            # ---------------- per-iteration phases ----------------
            xsrc = {"r": xb["r"], "q": xb["i"]}

            def emit_table(it, tb, td, ps_it, cast_act=False):
                bj = 0 if tb == "r" else 1  # column half that carries bias
                for i in range(8):
                    tst8 = bigp.tile([P, 2 * C], f8, tag="tst",
                                     name="tst8", bufs=4)
                    for j in range(2):
                        pst = ps_it.tile([P, C], f32, space="PSUM",
                                         tag="pst", name="pst", bufs=2)
                        for k in range(4):
                            nc.tensor.matmul(
                                pst[:],
                                xsrc[tb][k][:, i * P:(i + 1) * P],
                                Wc[tb][:, k, j * C:(j + 1) * C],
                                start=(k == 0), stop=(k == 3 and j != bj))
                        if j == bj:
                            nc.tensor.matmul(
                                pst[:], ones_row_bf[:],
                                brow[tb][:], start=False, stop=True,
                                skip_group_check=True)
                        # PSUM -> fp8 casts split across ACT and DVE
                        if j == 0 or cast_act:
                            nc.scalar.activation(
                                tst8[:, j * C:(j + 1) * C], pst[:], AF.Copy)
                        else:
                            nc.vector.tensor_copy(
                                tst8[:, j * C:(j + 1) * C], pst[:])
                    nc.sync.dma_start(td[i * P:(i + 1) * P, :], tst8[:])

            def gather_phase(it, ictx, ps_it, td_r, td_q):
                dap = ictx.enter_context(
                    tc.tile_pool(name=f"dabs{it}", bufs=4))
                psS_p = ictx.enter_context(
                    tc.tile_pool(name=f"psS{it}", bufs=1, space="PSUM"))
                ps_S = {q: psS_p.tile([64, C], f32, space="PSUM",
                                      tag=f"S{q}", name=f"S{q}_{it}")
                        for q in ("lin_r", "abs_r", "lin_i", "abs_i")}
                gt = {}

                def issue(ch):
                    g1t = dap.tile([P, 8, 2 * C], f8, tag="g1",
                                   name="g1t", bufs=5)
                    nc.gpsimd.dma_gather(
                        out_ap=g1t[:], in_ap=td_r[:],
                        idxs_ap=eidx_t[ch // 2][:, ch % 2, 0:64],
                        num_idxs=ECH, num_idxs_reg=ECH,
                        elem_size=2 * C)
                    g2t = dap.tile([P, 8, 2 * C], f8, tag="g2",
                                   name="g2t", bufs=5)
                    nc.gpsimd.dma_gather(
                        out_ap=g2t[:], in_ap=td_q[:],
                        idxs_ap=eidx_t[ch // 2][:, ch % 2, 64:128],
                        num_idxs=ECH, num_idxs_reg=ECH,
                        elem_size=2 * C)
                    gt[ch] = (g1t, g2t)

                # per-direction counts for PSUM accumulation start/stop flags
                n_emitted = {"lin_r": 0, "lin_i": 0, "abs_r": 0, "abs_i": 0}
                deferred = []

                def mm_reduce(kind, dirn, src):
                    k = n_emitted[f"{kind}_{dirn}"]
                    n_emitted[f"{kind}_{dirn}"] += 1
                    first = k == 0
                    last = k == NCH - 1
                    for s in range(4):
                        nc.tensor.matmul(
                            ps_S[f"{kind}_{dirn}"][:], ones64[:],
                            src[:, 2 * s:2 * s + 2, :],
                            start=(first and s == 0),
                            stop=(last and s == 3),
                            perf_mode=mybir.MatmulPerfMode.DoubleRow,
                            skip_group_check=True)

                def sub_part(dirn, ga, gbuf, lo, pool):
                    tg = "p" if pool else ""
                    dd = dap.tile([P, 8, C], f8, tag=f"{tg}dd",
                                  name="dd", bufs=4 if pool else 3)
                    seng = nc.gpsimd if pool else nc.vector
                    seng.tensor_tensor(
                        out=dd[:], in0=ga[:, :, lo:lo + C],
                        in1=gbuf[:, :, lo:lo + C], op=OP.subtract)
                    # fp8 DoubleRow: two edge rows per matmul; rows of
                    # the 64-row psum hold identical sums (read row 0)
                    mm_reduce("lin", dirn, dd)
                    return dd

                def abs_part(dirn, dd, pool):
                    tg = "p" if pool else ""
                    ad = dap.tile([P, 8, C], f8, tag=f"{tg}ad",
                                  name="ad", bufs=2)
                    if pool:
                        nc.gpsimd.tensor_scalar(
                            ad[:], dd[:], 0.0, None, op0=OP.abs_max)
                    else:
                        nc.scalar.activation(ad[:], dd[:], AF.Abs)
                    mm_reduce("abs", dirn, ad)

                for ch in range(min(LOOKAHEAD + 1, NCH)):
                    issue(ch)
                for ch in range(NCH):
                    g1t, g2t = gt[ch]
                    for di, (dirn, ga, gbuf, lo) in enumerate(
                            (("r", g1t, g2t, 0), ("i", g2t, g1t, C))):
                        u = ch * 2 + di
                        # GPSIMD work is deferred to the program tail so it
                        # never sits ahead of later desc-gen in Pool's queue
                        if u in POOL_SUB_UNITS:
                            deferred.append(("subabs", dirn, ga, gbuf, lo))
                        else:
                            dd = sub_part(dirn, ga, gbuf, lo, False)
                            if u in POOL_ABS_UNITS:
                                deferred.append(("abs", dirn, dd, None, None))
                            else:
                                abs_part(dirn, dd, False)
                    nxt = ch + LOOKAHEAD + 1
                    if nxt < NCH:
                        issue(nxt)
                for kind, dirn, a, b, lo in deferred:
                    if kind == "subabs":
                        dd = sub_part(dirn, a, b, lo, True)
                        abs_part(dirn, dd, False)
                    else:
                        abs_part(dirn, a, True)
                # S rows, AllReduce, chunked readback
                arin = dram.tile([2, C], f32, tag=f"arin{it}", name=f"arin{it}")
                arout = dram.tile([2, C], f32, tag=f"arout{it}",
                                  name=f"arout{it}")
                for row, dirn in ((0, "r"), (1, "i")):
                    t1r = dap.tile([1, C], f32, tag="t1r", name="t1r")
                    t2r = dap.tile([1, C], f32, tag="t2r", name="t2r")
                    nc.vector.tensor_scalar(t1r[:],
                                            ps_S[f"lin_{dirn}"][0:1, :],
                                            SC_LIN, None, op0=OP.mult)
                    nc.vector.tensor_scalar(t2r[:],
                                            ps_S[f"abs_{dirn}"][0:1, :],
                                            SC_ABS, None, op0=OP.mult)
                    nc.vector.tensor_tensor(out=t1r[:], in0=t1r[:], in1=t2r[:],
                                            op=OP.add)
                    nc.sync.dma_start(arin[row:row + 1, :], t1r[:])
                if timing:
                    nc.gpsimd.dma_start(arout[:], arin[:])
                else:
                    nc.gpsimd.collective_compute(
                        "AllReduce", OP.add,
                        replica_groups=[[0, 1], [2, 3], [4, 5], [6, 7]],
                        ins=[arin.opt()], outs=[arout.opt()])
                cS = dap.tile([P, 8], f32, tag="cS", name="cS")
                for row in range(2):
                    nc.sync.dma_start(
                        cS[:, row * 4:(row + 1) * 4],
                        arout[row:row + 1, :].rearrange(
                            "one (c p) -> (one p) c", p=P))
                # SE MLP
                with tc.tile_pool(name=f"ps_se{it}", bufs=1,
                                  space="PSUM") as ps_se_p:
                    ps_h1 = ps_se_p.tile([32, 1], f32, space="PSUM",
                                         tag="ps_h1", name="ps_h1")
                    for j in range(8):
                        nc.tensor.matmul(ps_h1[:], wse1_sb[:, j, :],
                                         cS[:, j:j + 1],
                                         start=(j == 0), stop=(j == 7))
                    h1r = dap.tile([32, 1], f32, tag="h1r", name="h1r")
                    nc.vector.tensor_tensor(out=h1r[:], in0=ps_h1[:],
                                            in1=bse1_sb[:], op=OP.add)
                    h1b = dap.tile([32, 1], f32, tag="h1b", name="h1b")
                    nc.vector.tensor_scalar_mul(h1b[:], h1r[:], 0.01)
                    nc.vector.tensor_tensor(out=h1r[:], in0=h1r[:], in1=h1b[:],
                                            op=OP.max)
                    ps_gate = ps_se_p.tile([P, 4], f32, space="PSUM",
                                           tag="ps_gate", name="ps_gate")
                    for j in range(4):
                        nc.tensor.matmul(ps_gate[:, j:j + 1],
                                         wse2_sb[:, j * P:(j + 1) * P],
                                         h1r[:], start=True, stop=True,
                                         skip_group_check=True)
                    gpre = dap.tile([P, 4], f32, tag="gpre", name="gpre")
                    nc.vector.tensor_tensor(out=gpre[:], in0=ps_gate[:],
                                            in1=bse2_sb[:], op=OP.add)
                    gate = dap.tile([P, 4], f32, tag="gate", name="gate")
                    nc.scalar.activation(gate[:], gpre[:], AF.Sigmoid)
                    nc.vector.tensor_tensor(out=a_r[:], in0=a_r[:], in1=gate[:],
                                            op=OP.mult)
                    omg = dap.tile([P, 4], f32, tag="omg", name="omg")
                    nc.vector.tensor_scalar(omg[:], gate[:], -1.0, 1.0,
                                            op0=OP.mult, op1=OP.add)
                    nc.vector.tensor_tensor(out=a_i[:], in0=a_i[:], in1=omg[:],
                                            op=OP.mult)
                    # fold the new gate into the combined weights in place:
                    # W_t+1 = gate_t (x) W_t along the contraction channels
                    for tb, gv in (("r", gate), ("q", omg)):
                        for k in range(4):
                            nc.vector.tensor_scalar(
                                Wc[tb][:, k, :], Wc[tb][:, k, :],
                                gv[:, k:k + 1], None, op0=OP.mult)

            # main flow: complete the r pipeline (load/pool/norm/top-k)
            # while the ir raster is still streaming in, then run the i
            # pipeline; per-t staged indices let it0's first gathers start
            # while later tiles are still ranking.
            td_r0 = dram.tile([NT, 2 * C], f8, tag="Tr0", name="Tr0")
            td_q0 = dram.tile([NT, 2 * C], f8, tag="Tq0", name="Tq0")
            exd_t = [dram.tile([1, 4096], u16, tag=f"exd{t}", name=f"exd{t}")
                     for t in range(4)]
            stage1_mod("r", ccs=(0, 1, 2, 3))
            # ir loads issued now (pure DMA), consumed after r's top-k
            raw_i = []
            for cc in range(4):
                rw = s1.tile([P, 64, 64], bf16, tag="rawi", name="rawi",
                             bufs=4)
                nc.sync.dma_start(rw[:], mod_in["i"][cc * P:(cc + 1) * P])
                raw_i.append(rw)
            emit_weights_prep()

            s2_ctx = ExitStack()
            s2 = s2_ctx.enter_context(tc.tile_pool(name="s2", bufs=2))
            s2b = s2_ctx.enter_context(tc.tile_pool(name="s2b", bufs=1))
            ps_g_p = s2_ctx.enter_context(
                tc.tile_pool(name="ps_g", bufs=2, space="PSUM"))
            # normalized features: Gram of xbn is the cosine similarity,
            # so ranking by it directly equals ranking by -distance
            xbn = {m: [s2b.tile([P, NT], bf16, tag=f"xbn_{m}{k}",
                                name=f"xbn_{m}{k}") for k in range(4)]
                   for m in MODS}

            def topk_mod(m):
                ab = 0 if m == "r" else 1
                for k in range(4):
                    nc.vector.tensor_tensor(out=xbn[m][k][:],
                                            in0=xb[m][k][:],
                                            in1=Bn[m][:], op=OP.mult)
                for t in range(4):
                    nd = s2.tile([P, NT], f32, tag="nd", name="nd")
                    for h in range(2):
                        psg = ps_g_p.tile([P, C], f32, space="PSUM",
                                          tag="psg", name="psg")
                        for k in range(4):
                            nc.tensor.matmul(
                                psg[:],
                                xbn[m][k][:, t * P:(t + 1) * P],
                                xbn[m][k][:, h * C:(h + 1) * C],
                                start=(k == 0), stop=(k == 3))
                        nc.scalar.activation(nd[:, h * C:(h + 1) * C],
                                             psg[:], AF.Copy)
                    mx = s2.tile([P, 16], f32, tag="mx", name="mx")
                    nc.vector.max(out=mx[:, 0:8], in_=nd[:])
                    nc.vector.max_index(out=idx_mt[m][t][:, 0:8],
                                        in_max=mx[:, 0:8], in_values=nd[:])
                    nc.vector.match_replace(out=nd[:],
                                            in_to_replace=mx[:, 0:8],
                                            in_values=nd[:],
                                            imm_value=-1e30)
                    nc.vector.max(out=mx[:, 8:16], in_=nd[:])
                    nc.vector.max_index(out=idx_mt[m][t][:, 8:16],
                                        in_max=mx[:, 8:16], in_values=nd[:])
                    # stage the tile's edge list: chunks 2t/2t+1, side ab
                    moff = ab * 1024
                    for hf in range(2):
                        base = hf * 2048 + moff
                        dst = exd_t[t][0:1, base:base + 1024].rearrange(
                            "one (p k) -> (one p) k", p=64)
                        nc.sync.dma_start(
                            dst, idx_mt[m][t][hf * 64:(hf + 1) * 64, :])
                    if m == "i":
                        # both modalities staged: wrap to 16 partitions
                        # (strided once) and broadcast to all 8 stripes
                        srcidx = exd_t[t][0:1, :].bitcast(i16).rearrange(
                            "one (c q) -> (one q) c", q=16)
                        nc.sync.dma_start(estg_t[t][:], srcidx)
                        for s8 in range(8):
                            nc.sync.dma_start(
                                eidx_t[t][s8 * 16:(s8 + 1) * 16, :, :],
                                estg_t[t][:])

            topk_mod("r")
            emit_table(0, "r", td_r0, ps_it0, cast_act=True)
            stage1_mod("i", ccs=(0, 1, 2, 3), raws=raw_i)
            emit_table(0, "q", td_q0, ps_it0, cast_act=True)
            topk_mod("i")
            s2_ctx.close()
            s1_ctx.close()

            gather_phase(0, it0_ctx, ps_it0, td_r0, td_q0)
            it0_ctx.close()
            for it in range(1, iterations):
                ictx = ExitStack()
                ps_it = ictx.enter_context(
                    tc.tile_pool(name=f"psit{it}", bufs=2, space="PSUM"))
                td_r = dram.tile([NT, 2 * C], f8, tag=f"Tr{it}",
                                 name=f"Tr{it}")
                td_q = dram.tile([NT, 2 * C], f8, tag=f"Tq{it}",
                                 name=f"Tq{it}")
                emit_table(it, "r", td_r, ps_it)
                emit_table(it, "q", td_q, ps_it)
                gather_phase(it, ictx, ps_it, td_r, td_q)
                ictx.close()

            # ---------------- output ----------------
            with tc.tile_pool(name="s6", bufs=2) as s6:
                alpha = s6.tile([P, 4], f32, tag="alpha", name="alpha")
                beta = s6.tile([P, 4], f32, tag="beta", name="beta")
                nc.vector.tensor_scalar(alpha[:], a_r[:], gb[1][:, 0:1], None,
                                        op0=OP.mult)
                nc.vector.tensor_scalar(beta[:], a_i[:], gb[2][:, 0:1], None,
                                        op0=OP.mult)
                for cc in range(4):
                    t1 = s6.tile([P, HN], f32, tag="t1", name="t1")
                    t2 = s6.tile([P, HN], f32, tag="t2", name="t2")
                    nc.vector.tensor_scalar(t1[:], phalf["r"][cc][:],
                                            alpha[:, cc:cc + 1], None,
                                            op0=OP.mult)
                    nc.vector.scalar_tensor_tensor(
                        out=t2[:], in0=phalf["i"][cc][:],
                        scalar=beta[:, cc:cc + 1], in1=t1[:],
                        op0=OP.mult, op1=OP.add)
                    nc.vector.tensor_scalar_max(t2[:], t2[:], 0.0)
                    nc.sync.dma_start(out_t[cc * P:(cc + 1) * P, :], t2[:])

    nc.compile()
    return nc


def _prepare_in_maps(rgb, ir, W_rgb_g, b_rgb_g, W_ir_g, b_ir_g,
                     W_se1, b_se1, W_se2, b_se2, gamma1, gamma2):
    f32 = np.float32
    common = {
        "wrgb": np.ascontiguousarray(W_rgb_g, f32),
        "wir": np.ascontiguousarray(W_ir_g, f32),
        "brgb": np.ascontiguousarray(b_rgb_g, f32).reshape(1, C),
        "bir": np.ascontiguousarray(b_ir_g, f32).reshape(1, C),
        "wse1": np.ascontiguousarray(W_se1, f32),
        "bse1": np.ascontiguousarray(b_se1, f32).reshape(1, 32),
        "wse2": np.ascontiguousarray(W_se2, f32),
        "bse2": np.ascontiguousarray(b_se2, f32).reshape(1, C),
        "g1": np.asarray(gamma1, f32).reshape(1, 1),
        "g2": np.asarray(gamma2, f32).reshape(1, 1),
    }
    in_maps = []
    for core in range(N_CORES):
        s, hh = core // 2, core % 2
        import ml_dtypes
        bf = ml_dtypes.bfloat16
        r = np.asarray(rgb[s], f32)
        i = np.asarray(ir[s], f32)
        if hh:
            r = np.roll(r, -32, axis=1)
            i = np.roll(i, -32, axis=1)
        m = dict(common)
        m["rgb"] = np.ascontiguousarray(r).astype(bf)
        m["ir"] = np.ascontiguousarray(i).astype(bf)
        in_maps.append(m)
    return in_maps


def _make_runner(nc):
    """Cached replica of bass2jax.run_bass_via_pjrt's multi-core branch so
    repeated kernel() calls skip jit retracing."""
    import jax
    import concourse.mybir as mybir
    from concourse import bass2jax as b2j
    from jax.experimental.shard_map import shard_map
    from jax.sharding import Mesh, PartitionSpec

    b2j.install_neuronx_cc_hook()

    partition_name = (nc.partition_id_tensor.name
                      if nc.partition_id_tensor else None)
    in_names, out_names, out_avals, zero_outs = [], [], [], []
    for alloc in nc.m.functions[0].allocations:
        if not isinstance(alloc, mybir.MemoryLocationSet):
            continue
        name = alloc.memorylocations[0].name
        if alloc.kind == "ExternalInput":
            if name != partition_name:
                in_names.append(name)
        elif alloc.kind == "ExternalOutput":
            shape = tuple(alloc.tensor_shape)
            np_dt = mybir.dt.np(alloc.dtype)
            out_names.append(name)
            out_avals.append(jax.core.ShapedArray(shape, np_dt))
            zero_outs.append(np.zeros(shape, np_dt))

    n_params = len(in_names)
    n_outs = len(out_names)
    all_in_names = list(in_names) + list(out_names)
    if partition_name is not None:
        all_in_names.append(partition_name)
    donate = tuple(range(n_params, n_params + n_outs))

    def _body(*args):
        operands = list(args)
        if partition_name is not None:
            operands.append(b2j.partition_id_tensor())
        outs = b2j._bass_exec_p.bind(
            *operands,
            out_avals=tuple(out_avals),
            in_names=tuple(all_in_names),
            out_names=tuple(out_names),
            lowering_input_output_aliases=(),
            sim_require_finite=True,
            sim_require_nnan=True,
            nc=nc,
        )
        return tuple(outs)

    devices = jax.devices()[:N_CORES]
    mesh = Mesh(np.asarray(devices), ("core",))
    in_specs = (PartitionSpec("core"),) * (n_params + n_outs)
    out_specs = (PartitionSpec("core"),) * n_outs
    sharded = jax.jit(
        shard_map(_body, mesh=mesh, in_specs=in_specs, out_specs=out_specs,
                  check_rep=False),
        donate_argnums=donate, keep_unused=True)
    concat_zeros = [np.zeros((N_CORES * z.shape[0], *z.shape[1:]), z.dtype)
                    for z in zero_outs]

    def run(in_maps):
        concat_in = [
            np.concatenate([np.asarray(in_maps[c][nm])
                            for c in range(N_CORES)], axis=0)
            for nm in in_names
        ]
        out_arrs = sharded(*concat_in, *[z.copy() for z in concat_zeros])
        return [
            {nm: np.asarray(out_arrs[i]).reshape(
                N_CORES, *out_avals[i].shape)[c]
             for i, nm in enumerate(out_names)}
            for c in range(N_CORES)
        ]

    return run


def kernel(rgb, ir, W_rgb_g, b_rgb_g, W_ir_g, b_ir_g,
           W_se1, b_se1, W_se2, b_se2, gamma1, gamma2,
           gnn_iterations, k):
    iterations = int(gnn_iterations)
    assert int(k) == KNN, f"kernel hardcodes k=16, got {k}"
    if iterations not in _CACHE:
        nc = _build(iterations)
        _CACHE[iterations] = _make_runner(nc)
    run = _CACHE[iterations]

    in_maps = _prepare_in_maps(rgb, ir, W_rgb_g, b_rgb_g, W_ir_g, b_ir_g,
                               W_se1, b_se1, W_se2, b_se2, gamma1, gamma2)
    results = run(in_maps)

    out = np.empty((4, C, 32, 32), np.float32)
    for s in range(4):
        lo = results[2 * s]["out"].reshape(C, 16, 32)
        hi = results[2 * s + 1]["out"].reshape(C, 16, 32)
        out[s] = np.concatenate([lo, hi], axis=1)
    return out


# revision 42
# speedup vs baseline: 1.1207x; 1.0062x over previous
"""Trainium2 Bass kernel for nn_FCN8sAtOnceMultiGnn2 (gnn_message_passing).

Strategy (8 NeuronCores; sample s = core//2, node-half = core%2):
  The GNN messages only feed a per-(sample,channel) SE gate: m_r/m_i are
  consumed by a full mean over nodes, so per iteration we only need
    S[c] = sum_edges lrelu(P[r_e,c] - Q[q_e,c] + b_c)
  where P/Q are per-sample tables h @ W (h = gate-scaled pooled features).
  The final output is relu(g1*prod(gate)*rgb_pooled + g2*prod(1-gate)*ir_pooled).

  Per core: maxpool -> bf16 Gram -> top-16 via DVE max8/max_index/match_replace
  -> edge lists -> per iteration: scale weights by accumulated gate products,
  compute combined tables T_r=[Wr1+Wr2 | Wi2] (rgb nodes), T_q=[Wr2 | Wi1+Wi2]
  (ir nodes) on the PE (+bias), cast fp8, write each to its own DRAM tensor
  (so a-side gathers only depend on T_r and can start while T_q is still
  emitting), dma_gather rows at the edge indices, d = sub (DVE/GPSIMD per a
  static schedule), |d| = Abs (ACT/DVE), reduce per channel with fp8
  DoubleRow ones-matmuls on PE accumulating in PSUM
  (lrelu sum = .505*sum(d)+.495*sum|d|), pairwise AllReduce the [2,512]
  partial sums, SE MLP -> gate. Host reassembles halves.
"""
import sys

sys.path.insert(0, "/opt/trn_rl_repo")

import numpy as np

_CACHE = {}

P = 128
C = 512          # channels
NT = 1024        # nodes per sample (32*32 after pool)
HN = 512         # nodes per core (half sample)
KNN = 16
E = HN * KNN     # 8192 edges per core per direction
NCH = 8          # gather chunks per iteration (per side)
ECH = E // NCH   # 1024 edge indices per gather
N_CORES = 8
LOOKAHEAD = 2    # chunks of gather issued ahead of elementwise work

# elementwise engine schedule: 16 units per iteration = (chunk, dirn)
# unit id u = ch*2 + dirn
POOL_SUB_UNITS = frozenset({5, 9, 13})          # subs on GPSIMD (rest DVE)
POOL_ABS_UNITS = frozenset()                    # (tensor_scalar invalid on Pool)
DVE_ABS_UNITS = frozenset()                     # (abs_max tensor_scalar is
                                                #  not a valid real-DVE op)


def _build(iterations: int, timing: bool = False):
    from contextlib import ExitStack

    import concourse.bacc as bacc
    import concourse.bass as bass
    import concourse.mybir as mybir
    import concourse.tile as tile

    dt = mybir.dt
    f32, bf16, i16, u16, f8 = (dt.float32, dt.bfloat16, dt.int16, dt.uint16,
                               dt.float8e4)
    AF = mybir.ActivationFunctionType
    OP = mybir.AluOpType
    SC_LIN = 0.505 / float(NT * KNN)
    SC_ABS = 0.495 / float(NT * KNN)

    nc = bacc.Bacc("TRN2", target_bir_lowering=False, debug=False,
                   num_devices=1 if timing else N_CORES)

    rgb_in = nc.dram_tensor("rgb", [C, 64, 64], bf16, kind="ExternalInput")
    ir_in = nc.dram_tensor("ir", [C, 64, 64], bf16, kind="ExternalInput")
    wrgb_in = nc.dram_tensor("wrgb", [2 * C, C], f32, kind="ExternalInput")
    wir_in = nc.dram_tensor("wir", [2 * C, C], f32, kind="ExternalInput")
    brgb_in = nc.dram_tensor("brgb", [1, C], f32, kind="ExternalInput")
    bir_in = nc.dram_tensor("bir", [1, C], f32, kind="ExternalInput")
    wse1_in = nc.dram_tensor("wse1", [2 * C, 32], f32, kind="ExternalInput")
    bse1_in = nc.dram_tensor("bse1", [1, 32], f32, kind="ExternalInput")
    wse2_in = nc.dram_tensor("wse2", [32, C], f32, kind="ExternalInput")
    bse2_in = nc.dram_tensor("bse2", [1, C], f32, kind="ExternalInput")
    g1_in = nc.dram_tensor("g1", [1, 1], f32, kind="ExternalInput")
    g2_in = nc.dram_tensor("g2", [1, 1], f32, kind="ExternalInput")
    out_t = nc.dram_tensor("out", [C, HN], f32, kind="ExternalOutput")

    MODS = ("r", "i")
    mod_in = {"r": rgb_in, "i": ir_in}

    with tile.TileContext(nc) as tc:
        with (
            tc.tile_pool(name="persist", bufs=1) as pp,
            tc.tile_pool(name="big", bufs=2) as bigp,
            tc.tile_pool(name="dram", bufs=1, space="DRAM") as dram,
        ):
            # ---------------- constants / persistent tiles ----------------
            ones_bf = pp.tile([P, 1], bf16, tag="ones_bf")
            nc.vector.memset(ones_bf[:], 1.0)
            ones64 = pp.tile([P, 2, 64], f8, tag="ones64")
            nc.vector.memset(ones64[:], 1.0)
            ones_row = pp.tile([1, P], f32, tag="ones_row")
            nc.vector.memset(ones_row[:], 1.0)
            ones_row_bf = pp.tile([1, P], bf16, tag="ones_row_bf")
            nc.vector.memset(ones_row_bf[:], 1.0)

            xb = {m: [pp.tile([P, NT], bf16, tag=f"xb_{m}{cc}", name=f"xb_{m}{cc}")
                      for cc in range(4)] for m in MODS}
            phalf = {m: [pp.tile([P, HN], bf16, tag=f"ph_{m}{cc}", name=f"ph_{m}{cc}")
                         for cc in range(4)] for m in MODS}
            rn = {m: pp.tile([1, NT], f32, tag=f"rn_{m}", name=f"rn_{m}")
                  for m in MODS}
            Bn = {m: pp.tile([P, NT], bf16, tag=f"Bn_{m}", name=f"Bn_{m}")
                  for m in MODS}
            idx_mt = {m: [pp.tile([P, KNN], u16, tag=f"ix_{m}{t}", name=f"ix_{m}{t}")
                          for t in range(4)] for m in MODS}
            eidx_t = [pp.tile([P, 2, 128], i16, tag=f"eix{t}", name=f"eix{t}")
                      for t in range(4)]
            estg_t = [pp.tile([16, 2, 128], i16, tag=f"estg{t}",
                              name=f"estg{t}") for t in range(4)]
            Wc = {"r": pp.tile([P, 4, 2 * C], bf16, tag="Wc_r", name="Wc_r"),
                  "q": pp.tile([P, 4, 2 * C], bf16, tag="Wc_q", name="Wc_q")}
            # bias rows for the PE bias-matmul: r-table biases cols 0:C,
            # q-table biases cols C:2C; the other half has zero bias
            brow = {"r": pp.tile([1, C], bf16, tag="brow_r", name="brow_r"),
                    "q": pp.tile([1, C], bf16, tag="brow_q", name="brow_q")}
            wse1_sb = pp.tile([P, 8, 32], f32, tag="wse1", name="wse1")
            bse1_sb = pp.tile([32, 1], f32, tag="bse1", name="bse1")
            wse2_sb = pp.tile([32, C], f32, tag="wse2", name="wse2")
            bse2_sb = pp.tile([P, 4], f32, tag="bse2", name="bse2")
            gb = {1: pp.tile([P, 1], f32, tag="gb1", name="gb1"),
                  2: pp.tile([P, 1], f32, tag="gb2", name="gb2")}
            a_r = pp.tile([P, 4], f32, tag="a_r", name="a_r")
            a_i = pp.tile([P, 4], f32, tag="a_i", name="a_i")
            nc.vector.memset(a_r[:], 1.0)
            nc.vector.memset(a_i[:], 1.0)

            # ---------------- weights / SE / bias prep ----------------
            def emit_weights_prep():
                with (
                    tc.tile_pool(name="s4", bufs=1) as s4,
                    tc.tile_pool(name="ps_c", bufs=2, space="PSUM") as ps_c_p,
                ):
                    def wload(dst, src_t, lohi):
                        nc.sync.dma_start(
                            dst[:],
                            src_t[lohi * C:(lohi + 1) * C, :].rearrange(
                                "(k p) c -> p k c", p=P))
                    wa = s4.tile([P, 4, C], f32, tag="wa", name="wa")
                    wb = s4.tile([P, 4, C], f32, tag="wb", name="wb")
                    wload(wa, wrgb_in, 1)
                    wload(wb, wrgb_in, 0)
                    for k in range(4):
                        nc.gpsimd.tensor_tensor(out=Wc["r"][:, k, 0:C],
                                                in0=wb[:, k, :],
                                                in1=wa[:, k, :], op=OP.add)
                        nc.scalar.activation(Wc["q"][:, k, 0:C], wa[:, k, :],
                                             AF.Copy)
                    wb2 = s4.tile([P, 4, C], f32, tag="wb", name="wb2")
                    wload(wb2, wir_in, 1)
                    for k in range(4):
                        nc.scalar.activation(Wc["r"][:, k, C:2 * C],
                                             wb2[:, k, :], AF.Copy)
                    wa2 = s4.tile([P, 4, C], f32, tag="wa", name="wa2")
                    wload(wa2, wir_in, 0)
                    for k in range(4):
                        nc.gpsimd.tensor_tensor(out=Wc["q"][:, k, C:2 * C],
                                                in0=wa2[:, k, :],
                                                in1=wb2[:, k, :], op=OP.add)
                    brow32 = s4.tile([1, C], f32, tag="brow32", name="brow32")
                    for nm, src_b in (("r", brgb_in), ("q", bir_in)):
                        nc.sync.dma_start(brow32[:], src_b[:])
                        nc.vector.tensor_copy(brow[nm][:], brow32[:])
                    nc.sync.dma_start(
                        wse1_sb[:],
                        wse1_in[:].rearrange("(k p) n -> p k n", p=P))

                    nc.sync.dma_start(bse1_sb[:], bse1_in[:].rearrange("a b -> b a"))
                    nc.sync.dma_start(wse2_sb[:], wse2_in[:])
                    nc.sync.dma_start(
                        bse2_sb[:],
                        bse2_in[:].rearrange("one (c p) -> (one p) c", p=P))
                    for gi, gsrc in ((1, g1_in), (2, g2_in)):
                        grow = s4.tile([1, 1], f32, tag="grow", name="grow")
                        nc.sync.dma_start(grow[:], gsrc[:])
                        psg2 = ps_c_p.tile([P, 1], f32, space="PSUM", tag="psg2",
                                           name="psg2")
                        nc.tensor.matmul(psg2[:], ones_row[:], grow[:],
                                         start=True, stop=True)
                        nc.vector.tensor_copy(gb[gi][:], psg2[:])

            # ---------------- stage 1 (per modality) ----------------
            drow_d_map = {}
            it0_ctx = ExitStack()
            it0_psit_ctx = ExitStack()
            ps_it0 = it0_psit_ctx.enter_context(
                tc.tile_pool(name="psit0", bufs=2, space="PSUM"))
            s1_ctx = ExitStack()
            s1 = s1_ctx.enter_context(tc.tile_pool(name="s1", bufs=1))
            ps_ss_p = s1_ctx.enter_context(
                tc.tile_pool(name="ps_ss", bufs=1, space="PSUM"))

            ps_ss_map = {}

            def stage1_mod(m, ccs=(0, 1, 2, 3), tail=True, raws=None):
                if m not in ps_ss_map:
                    ps_ss_map[m] = [ps_ss_p.tile([1, C], f32, space="PSUM",
                                                 tag=f"ss{h}",
                                                 name=f"ss{m}{h}")
                                    for h in range(2)]
                ps_ss = ps_ss_map[m]
                for cc in ccs:
                    if raws is None:
                        raw = s1.tile([P, 64, 64], bf16, tag="raw",
                                      name="raw", bufs=3)
                        nc.sync.dma_start(raw[:],
                                          mod_in[m][cc * P:(cc + 1) * P])
                    else:
                        raw = raws[cc]
                    h1 = s1.tile([P, 32, 64], bf16, tag="h1", name="h1",
                                 bufs=2)
                    nc.vector.tensor_tensor(out=h1[:], in0=raw[:, 0::2, :],
                                            in1=raw[:, 1::2, :], op=OP.max)
                    pf = s1.tile([P, 32, 32], bf16, tag="pf", name="pf",
                                 bufs=2)
                    nc.vector.tensor_tensor(out=pf[:], in0=h1[:, :, 0::2],
                                            in1=h1[:, :, 1::2], op=OP.max)
                    pff = pf.rearrange("p a b -> p (a b)")
                    nc.scalar.activation(xb[m][cc][:], pff, AF.Copy)
                    nc.scalar.activation(phalf[m][cc][:], pff[:, 0:HN], AF.Copy)
                    sq = s1.tile([P, NT], bf16, tag="sq", name="sq", bufs=2)
                    nc.scalar.activation(sq[:], pff, AF.Square)
                    for h in range(2):
                        nc.tensor.matmul(ps_ss[h][:], ones_bf[:],
                                         sq[:, h * C:(h + 1) * C],
                                         start=(cc == 0), stop=(cc == 3))
                if not tail:
                    return
                srow = s1.tile([1, NT], f32, tag="srow", name="srow")
                for h in range(2):
                    nc.scalar.activation(srow[:, h * C:(h + 1) * C],
                                         ps_ss[h][:], AF.Sqrt)
                nc.vector.tensor_scalar_max(srow[:], srow[:], 1e-12)
                nc.vector.reciprocal(rn[m][:], srow[:])
                rnb = s1.tile([1, NT], bf16, tag="rnb", name="rnb")
                nc.vector.tensor_copy(rnb[:], rn[m][:])
                for h in range(2):
                    psb = ps_ss_p.tile([P, C], f32, space="PSUM", tag="psbn",
                                       name=f"psbn_{m}{h}", bufs=2)
                    nc.tensor.matmul(psb[:], ones_row_bf[:],
                                     rnb[:, h * C:(h + 1) * C],
                                     start=True, stop=True)
                    nc.scalar.activation(Bn[m][:, h * C:(h + 1) * C],
                                         psb[:], AF.Copy)

            # ---------------- per-iteration phases ----------------
            xsrc = {"r": xb["r"], "q": xb["i"]}

            def emit_table(it, tb, td, ps_it, cast_act=False, pst_bufs=2):
                bj = 0 if tb == "r" else 1  # column half that carries bias
                for i in range(8):
                    tst8 = bigp.tile([P, 2 * C], f8, tag="tst",
                                     name="tst8", bufs=4)
                    for j in range(2):
                        pst = ps_it.tile([P, C], f32, space="PSUM",
                                         tag="pst", name="pst", bufs=pst_bufs)
                        for k in range(4):
                            nc.tensor.matmul(
                                pst[:],
                                xsrc[tb][k][:, i * P:(i + 1) * P],
                                Wc[tb][:, k, j * C:(j + 1) * C],
                                start=(k == 0), stop=(k == 3 and j != bj))
                        if j == bj:
                            nc.tensor.matmul(
                                pst[:], ones_row_bf[:],
                                brow[tb][:], start=False, stop=True,
                                skip_group_check=True)
                        # PSUM -> fp8 casts: alternate engines by block so
                        # consecutive blocks' casts pipeline in parallel
                        if cast_act or i % 2 == 0:
                            nc.scalar.activation(
                                tst8[:, j * C:(j + 1) * C], pst[:], AF.Copy)
                        else:
                            nc.vector.tensor_copy(
                                tst8[:, j * C:(j + 1) * C], pst[:])
                    nc.sync.dma_start(td[i * P:(i + 1) * P, :], tst8[:])

            def gather_phase(it, ictx, ps_it, td_r, td_q):
                dap = ictx.enter_context(
                    tc.tile_pool(name=f"dabs{it}", bufs=4))
                psS_p = ictx.enter_context(
                    tc.tile_pool(name=f"psS{it}", bufs=1, space="PSUM"))
                ps_S = {q: psS_p.tile([64, C], f32, space="PSUM",
                                      tag=f"S{q}", name=f"S{q}_{it}")
                        for q in ("lin_r", "abs_r", "lin_i", "abs_i")}
                gt = {}

                def issue(ch):
                    g1t = dap.tile([P, 8, 2 * C], f8, tag="g1",
                                   name="g1t", bufs=5)
                    nc.gpsimd.dma_gather(
                        out_ap=g1t[:], in_ap=td_r[:],
                        idxs_ap=eidx_t[ch // 2][:, ch % 2, 0:64],
                        num_idxs=ECH, num_idxs_reg=ECH,
                        elem_size=2 * C)
                    g2t = dap.tile([P, 8, 2 * C], f8, tag="g2",
                                   name="g2t", bufs=5)
                    nc.gpsimd.dma_gather(
                        out_ap=g2t[:], in_ap=td_q[:],
                        idxs_ap=eidx_t[ch // 2][:, ch % 2, 64:128],
                        num_idxs=ECH, num_idxs_reg=ECH,
                        elem_size=2 * C)
                    gt[ch] = (g1t, g2t)

                # per-direction counts for PSUM accumulation start/stop flags
                n_emitted = {"lin_r": 0, "lin_i": 0, "abs_r": 0, "abs_i": 0}
                deferred = []

                def mm_reduce(kind, dirn, src):
                    k = n_emitted[f"{kind}_{dirn}"]
                    n_emitted[f"{kind}_{dirn}"] += 1
                    first = k == 0
                    last = k == NCH - 1
                    for s in range(4):
                        nc.tensor.matmul(
                            ps_S[f"{kind}_{dirn}"][:], ones64[:],
                            src[:, 2 * s:2 * s + 2, :],
                            start=(first and s == 0),
                            stop=(last and s == 3),
                            perf_mode=mybir.MatmulPerfMode.DoubleRow,
                            skip_group_check=True)

                def sub_part(dirn, ga, gbuf, lo, pool):
                    tg = "p" if pool else ""
                    dd = dap.tile([P, 8, C], f8, tag=f"{tg}dd",
                                  name="dd", bufs=4 if pool else 3)
                    seng = nc.gpsimd if pool else nc.vector
                    seng.tensor_tensor(
                        out=dd[:], in0=ga[:, :, lo:lo + C],
                        in1=gbuf[:, :, lo:lo + C], op=OP.subtract)
                    # fp8 DoubleRow: two edge rows per matmul; rows of
                    # the 64-row psum hold identical sums (read row 0)
                    mm_reduce("lin", dirn, dd)
                    return dd

                def abs_part(dirn, dd, pool):
                    tg = "p" if pool else ""
                    ad = dap.tile([P, 8, C], f8, tag=f"{tg}ad",
                                  name="ad", bufs=2)
                    if pool:
                        nc.gpsimd.tensor_scalar(
                            ad[:], dd[:], 0.0, None, op0=OP.abs_max)
                    else:
                        nc.scalar.activation(ad[:], dd[:], AF.Abs)
                    mm_reduce("abs", dirn, ad)

                for ch in range(min(LOOKAHEAD + 1, NCH)):
                    issue(ch)
                for ch in range(NCH):
                    g1t, g2t = gt[ch]
                    for di, (dirn, ga, gbuf, lo) in enumerate(
                            (("r", g1t, g2t, 0), ("i", g2t, g1t, C))):
                        u = ch * 2 + di
                        # GPSIMD work is deferred to the program tail so it
                        # never sits ahead of later desc-gen in Pool's queue
                        if u in POOL_SUB_UNITS:
                            deferred.append(("subabs", dirn, ga, gbuf, lo))
                        else:
                            dd = sub_part(dirn, ga, gbuf, lo, False)
                            if u in POOL_ABS_UNITS:
                                deferred.append(("abs", dirn, dd, None, None))
                            else:
                                abs_part(dirn, dd, False)
                    nxt = ch + LOOKAHEAD + 1
                    if nxt < NCH:
                        issue(nxt)
                for kind, dirn, a, b, lo in deferred:
                    if kind == "subabs":
                        dd = sub_part(dirn, a, b, lo, True)
                        abs_part(dirn, dd, False)
                    else:
                        abs_part(dirn, a, True)
                # S rows, AllReduce, chunked readback
                arin = dram.tile([2, C], f32, tag=f"arin{it}", name=f"arin{it}")
                arout = dram.tile([2, C], f32, tag=f"arout{it}",
                                  name=f"arout{it}")
                for row, dirn in ((0, "r"), (1, "i")):
                    t1r = dap.tile([1, C], f32, tag="t1r", name="t1r")
                    t2r = dap.tile([1, C], f32, tag="t2r", name="t2r")
                    nc.vector.tensor_scalar(t1r[:],
                                            ps_S[f"lin_{dirn}"][0:1, :],
                                            SC_LIN, None, op0=OP.mult)
                    nc.vector.tensor_scalar(t2r[:],
                                            ps_S[f"abs_{dirn}"][0:1, :],
                                            SC_ABS, None, op0=OP.mult)
                    nc.vector.tensor_tensor(out=t1r[:], in0=t1r[:], in1=t2r[:],
                                            op=OP.add)
                    nc.sync.dma_start(arin[row:row + 1, :], t1r[:])
                if timing:
                    nc.gpsimd.dma_start(arout[:], arin[:])
                else:
                    nc.gpsimd.collective_compute(
                        "AllReduce", OP.add,
                        replica_groups=[[0, 1], [2, 3], [4, 5], [6, 7]],
                        ins=[arin.opt()], outs=[arout.opt()])
                cS = dap.tile([P, 8], f32, tag="cS", name="cS")
                for row in range(2):
                    nc.sync.dma_start(
                        cS[:, row * 4:(row + 1) * 4],
                        arout[row:row + 1, :].rearrange(
                            "one (c p) -> (one p) c", p=P))
                # SE MLP
                with tc.tile_pool(name=f"ps_se{it}", bufs=1,
                                  space="PSUM") as ps_se_p:
                    ps_h1 = ps_se_p.tile([32, 1], f32, space="PSUM",
                                         tag="ps_h1", name="ps_h1")
                    for j in range(8):
                        nc.tensor.matmul(ps_h1[:], wse1_sb[:, j, :],
                                         cS[:, j:j + 1],
                                         start=(j == 0), stop=(j == 7))
                    h1r = dap.tile([32, 1], f32, tag="h1r", name="h1r")
                    nc.vector.tensor_tensor(out=h1r[:], in0=ps_h1[:],
                                            in1=bse1_sb[:], op=OP.add)
                    h1b = dap.tile([32, 1], f32, tag="h1b", name="h1b")
                    nc.vector.tensor_scalar_mul(h1b[:], h1r[:], 0.01)
                    nc.vector.tensor_tensor(out=h1r[:], in0=h1r[:], in1=h1b[:],
                                            op=OP.max)
                    ps_gate = ps_se_p.tile([P, 4], f32, space="PSUM",
                                           tag="ps_gate", name="ps_gate")
                    for j in range(4):
                        nc.tensor.matmul(ps_gate[:, j:j + 1],
                                         wse2_sb[:, j * P:(j + 1) * P],
                                         h1r[:], start=True, stop=True,
                                         skip_group_check=True)
                    gpre = dap.tile([P, 4], f32, tag="gpre", name="gpre")
                    nc.vector.tensor_tensor(out=gpre[:], in0=ps_gate[:],
                                            in1=bse2_sb[:], op=OP.add)
                    gate = dap.tile([P, 4], f32, tag="gate", name="gate")
                    nc.scalar.activation(gate[:], gpre[:], AF.Sigmoid)
                    nc.vector.tensor_tensor(out=a_r[:], in0=a_r[:], in1=gate[:],
                                            op=OP.mult)
                    omg = dap.tile([P, 4], f32, tag="omg", name="omg")
                    nc.vector.tensor_scalar(omg[:], gate[:], -1.0, 1.0,
                                            op0=OP.mult, op1=OP.add)
                    nc.vector.tensor_tensor(out=a_i[:], in0=a_i[:], in1=omg[:],
                                            op=OP.mult)
                    # fold the new gate into the combined weights in place:
                    # W_t+1 = gate_t (x) W_t along the contraction channels
                    # (not needed after the last iteration)
                    if it < iterations - 1:
                        for tb, gv in (("r", gate), ("q", omg)):
                            for k in range(4):
                                nc.vector.tensor_scalar(
                                    Wc[tb][:, k, :], Wc[tb][:, k, :],
                                    gv[:, k:k + 1], None, op0=OP.mult)

            # main flow: complete the r pipeline (load/pool/norm/top-k)
            # while the ir raster is still streaming in, then run the i
            # pipeline; per-t staged indices let it0's first gathers start
            # while later tiles are still ranking.
            td_r0 = dram.tile([NT, 2 * C], f8, tag="Tr0", name="Tr0")
            td_q0 = dram.tile([NT, 2 * C], f8, tag="Tq0", name="Tq0")
            exd_t = [dram.tile([1, 4096], u16, tag=f"exd{t}", name=f"exd{t}")
                     for t in range(4)]
            stage1_mod("r", ccs=(0, 1, 2, 3))
            # ir loads issued now (pure DMA), consumed after r's top-k
            raw_i = []
            for cc in range(4):
                rw = s1.tile([P, 64, 64], bf16, tag="rawi", name="rawi",
                             bufs=4)
                nc.sync.dma_start(rw[:], mod_in["i"][cc * P:(cc + 1) * P])
                raw_i.append(rw)
            emit_weights_prep()

            s2_ctx = ExitStack()
            s2 = s2_ctx.enter_context(tc.tile_pool(name="s2", bufs=2))
            s2b = s2_ctx.enter_context(tc.tile_pool(name="s2b", bufs=1))
            ps_g_p = s2_ctx.enter_context(
                tc.tile_pool(name="ps_g", bufs=2, space="PSUM"))
            # normalized features: Gram of xbn is the cosine similarity,
            # so ranking by it directly equals ranking by -distance
            xbn = {m: [s2b.tile([P, NT], bf16, tag=f"xbn_{m}{k}",
                                name=f"xbn_{m}{k}") for k in range(4)]
                   for m in MODS}

            def topk_mod(m):
                ab = 0 if m == "r" else 1
                for k in range(4):
                    nc.vector.tensor_tensor(out=xbn[m][k][:],
                                            in0=xb[m][k][:],
                                            in1=Bn[m][:], op=OP.mult)
                for t in range(4):
                    nd = s2.tile([P, NT], f32, tag="nd", name="nd")
                    for h in range(2):
                        psg = ps_g_p.tile([P, C], f32, space="PSUM",
                                          tag="psg", name="psg")
                        for k in range(4):
                            nc.tensor.matmul(
                                psg[:],
                                xbn[m][k][:, t * P:(t + 1) * P],
                                xbn[m][k][:, h * C:(h + 1) * C],
                                start=(k == 0), stop=(k == 3))
                        nc.scalar.activation(nd[:, h * C:(h + 1) * C],
                                             psg[:], AF.Copy)
                    mx = s2.tile([P, 16], f32, tag="mx", name="mx")
                    nc.vector.max(out=mx[:, 0:8], in_=nd[:])
                    nc.vector.max_index(out=idx_mt[m][t][:, 0:8],
                                        in_max=mx[:, 0:8], in_values=nd[:])
                    nc.vector.match_replace(out=nd[:],
                                            in_to_replace=mx[:, 0:8],
                                            in_values=nd[:],
                                            imm_value=-1e30)
                    nc.vector.max(out=mx[:, 8:16], in_=nd[:])
                    nc.vector.max_index(out=idx_mt[m][t][:, 8:16],
                                        in_max=mx[:, 8:16], in_values=nd[:])
                    # stage the tile's edge list: chunks 2t/2t+1, side ab
                    moff = ab * 1024
                    for hf in range(2):
                        base = hf * 2048 + moff
                        dst = exd_t[t][0:1, base:base + 1024].rearrange(
                            "one (p k) -> (one p) k", p=64)
                        nc.sync.dma_start(
                            dst, idx_mt[m][t][hf * 64:(hf + 1) * 64, :])
                    if m == "i":
                        # both modalities staged: wrap to 16 partitions
                        # (strided once) and broadcast to all 8 stripes
                        srcidx = exd_t[t][0:1, :].bitcast(i16).rearrange(
                            "one (c q) -> (one q) c", q=16)
                        nc.sync.dma_start(estg_t[t][:], srcidx)
                        for s8 in range(8):
                            nc.sync.dma_start(
                                eidx_t[t][s8 * 16:(s8 + 1) * 16, :, :],
                                estg_t[t][:])

            topk_mod("r")
            emit_table(0, "r", td_r0, ps_it0, cast_act=True)
            stage1_mod("i", ccs=(0, 1, 2, 3), raws=raw_i)
            emit_table(0, "q", td_q0, ps_it0, cast_act=True)
            topk_mod("i")
            s2_ctx.close()
            s1_ctx.close()

            it0_psit_ctx.close()
            gather_phase(0, it0_ctx, None, td_r0, td_q0)
            it0_ctx.close()
            for it in range(1, iterations):
                td_r = dram.tile([NT, 2 * C], f8, tag=f"Tr{it}",
                                 name=f"Tr{it}")
                td_q = dram.tile([NT, 2 * C], f8, tag=f"Tq{it}",
                                 name=f"Tq{it}")
                with tc.tile_pool(name=f"psit{it}", bufs=1,
                                  space="PSUM") as ps_it:
                    emit_table(it, "r", td_r, ps_it, pst_bufs=3)
                    emit_table(it, "q", td_q, ps_it, pst_bufs=3)
                ictx = ExitStack()
                gather_phase(it, ictx, None, td_r, td_q)
                ictx.close()

            # ---------------- output ----------------
            with tc.tile_pool(name="s6", bufs=2) as s6:
                alpha = s6.tile([P, 4], f32, tag="alpha", name="alpha")
                beta = s6.tile([P, 4], f32, tag="beta", name="beta")
                nc.vector.tensor_scalar(alpha[:], a_r[:], gb[1][:, 0:1], None,
                                        op0=OP.mult)
                nc.vector.tensor_scalar(beta[:], a_i[:], gb[2][:, 0:1], None,
                                        op0=OP.mult)
                for cc in range(4):
                    t1 = s6.tile([P, HN], f32, tag="t1", name="t1")
                    t2 = s6.tile([P, HN], f32, tag="t2", name="t2")
                    nc.vector.tensor_scalar(t1[:], phalf["r"][cc][:],
                                            alpha[:, cc:cc + 1], None,
                                            op0=OP.mult)
                    nc.vector.scalar_tensor_tensor(
                        out=t2[:], in0=phalf["i"][cc][:],
                        scalar=beta[:, cc:cc + 1], in1=t1[:],
                        op0=OP.mult, op1=OP.add)
                    nc.vector.tensor_scalar_max(t2[:], t2[:], 0.0)
                    nc.sync.dma_start(out_t[cc * P:(cc + 1) * P, :], t2[:])

    nc.compile()
    return nc


def _prepare_in_maps(rgb, ir, W_rgb_g, b_rgb_g, W_ir_g, b_ir_g,
                     W_se1, b_se1, W_se2, b_se2, gamma1, gamma2):
    f32 = np.float32
    common = {
        "wrgb": np.ascontiguousarray(W_rgb_g, f32),
        "wir": np.ascontiguousarray(W_ir_g, f32),
        "brgb": np.ascontiguousarray(b_rgb_g, f32).reshape(1, C),
        "bir": np.ascontiguousarray(b_ir_g, f32).reshape(1, C),
        "wse1": np.ascontiguousarray(W_se1, f32),
        "bse1": np.ascontiguousarray(b_se1, f32).reshape(1, 32),
        "wse2": np.ascontiguousarray(W_se2, f32),
        "bse2": np.ascontiguousarray(b_se2, f32).reshape(1, C),
        "g1": np.asarray(gamma1, f32).reshape(1, 1),
        "g2": np.asarray(gamma2, f32).reshape(1, 1),
    }
    in_maps = []
    for core in range(N_CORES):
        s, hh = core // 2, core % 2
        import ml_dtypes
        bf = ml_dtypes.bfloat16
        r = np.asarray(rgb[s], f32)
        i = np.asarray(ir[s], f32)
        if hh:
            r = np.roll(r, -32, axis=1)
            i = np.roll(i, -32, axis=1)
        m = dict(common)
        m["rgb"] = np.ascontiguousarray(r).astype(bf)
        m["ir"] = np.ascontiguousarray(i).astype(bf)
        in_maps.append(m)
    return in_maps


def _make_runner(nc):
    """Cached replica of bass2jax.run_bass_via_pjrt's multi-core branch so
    repeated kernel() calls skip jit retracing."""
    import jax
    import concourse.mybir as mybir
    from concourse import bass2jax as b2j
    from jax.experimental.shard_map import shard_map
    from jax.sharding import Mesh, PartitionSpec

    b2j.install_neuronx_cc_hook()

    partition_name = (nc.partition_id_tensor.name
                      if nc.partition_id_tensor else None)
    in_names, out_names, out_avals, zero_outs = [], [], [], []
    for alloc in nc.m.functions[0].allocations:
        if not isinstance(alloc, mybir.MemoryLocationSet):
            continue
        name = alloc.memorylocations[0].name
        if alloc.kind == "ExternalInput":
            if name != partition_name:
                in_names.append(name)
        elif alloc.kind == "ExternalOutput":
            shape = tuple(alloc.tensor_shape)
            np_dt = mybir.dt.np(alloc.dtype)
            out_names.append(name)
            out_avals.append(jax.core.ShapedArray(shape, np_dt))
            zero_outs.append(np.zeros(shape, np_dt))

    n_params = len(in_names)
    n_outs = len(out_names)
    all_in_names = list(in_names) + list(out_names)
    if partition_name is not None:
        all_in_names.append(partition_name)
    donate = tuple(range(n_params, n_params + n_outs))

    def _body(*args):
        operands = list(args)
        if partition_name is not None:
            operands.append(b2j.partition_id_tensor())
        outs = b2j._bass_exec_p.bind(
            *operands,
            out_avals=tuple(out_avals),
            in_names=tuple(all_in_names),
            out_names=tuple(out_names),
            lowering_input_output_aliases=(),
            sim_require_finite=True,
            sim_require_nnan=True,
            nc=nc,
        )
        return tuple(outs)

    devices = jax.devices()[:N_CORES]
    mesh = Mesh(np.asarray(devices), ("core",))
    in_specs = (PartitionSpec("core"),) * (n_params + n_outs)
    out_specs = (PartitionSpec("core"),) * n_outs
    sharded = jax.jit(
        shard_map(_body, mesh=mesh, in_specs=in_specs, out_specs=out_specs,
                  check_rep=False),
        donate_argnums=donate, keep_unused=True)
    concat_zeros = [np.zeros((N_CORES * z.shape[0], *z.shape[1:]), z.dtype)
                    for z in zero_outs]

    def run(in_maps):
        concat_in = [
            np.concatenate([np.asarray(in_maps[c][nm])
                            for c in range(N_CORES)], axis=0)
            for nm in in_names
        ]
        out_arrs = sharded(*concat_in, *[z.copy() for z in concat_zeros])
        return [
            {nm: np.asarray(out_arrs[i]).reshape(
                N_CORES, *out_avals[i].shape)[c]
             for i, nm in enumerate(out_names)}
            for c in range(N_CORES)
        ]

    return run


def kernel(rgb, ir, W_rgb_g, b_rgb_g, W_ir_g, b_ir_g,
           W_se1, b_se1, W_se2, b_se2, gamma1, gamma2,
           gnn_iterations, k):
    iterations = int(gnn_iterations)
    assert int(k) == KNN, f"kernel hardcodes k=16, got {k}"
    if iterations not in _CACHE:
        nc = _build(iterations)
        _CACHE[iterations] = _make_runner(nc)
    run = _CACHE[iterations]

    in_maps = _prepare_in_maps(rgb, ir, W_rgb_g, b_rgb_g, W_ir_g, b_ir_g,
                               W_se1, b_se1, W_se2, b_se2, gamma1, gamma2)
    results = run(in_maps)

    out = np.empty((4, C, 32, 32), np.float32)
    for s in range(4):
        lo = results[2 * s]["out"].reshape(C, 16, 32)
        hi = results[2 * s + 1]["out"].reshape(C, 16, 32)
        out[s] = np.concatenate([lo, hi], axis=1)
    return out
